# revision 5
# baseline (speedup 1.0000x reference)
"""Trainium2 Bass kernel for EpochedFutureFill: exact causal convolution
y[b,t] = sum_s filt[s] x[b,t-s], computed via a 65536-point FFT per row.

Strategy (data-parallel over 8 cores, 32 rows each):
  - Pack two real rows per complex signal: z = x_a + i*x_b. Since
    IFFT(S * FFT(z)) is linear over R^2, Re -> conv(x_a), Im -> conv(x_b).
  - 65536-pt FFT as 2-stage Cooley-Tukey (256 x 256) built from 256-pt DFT
    matmuls on the tensor engine; twiddle/spectrum pointwise multiplies on
    the vector engine (+ gpsimd for SBUF-side combines).
  - Matmul orientations chosen so no transposes are ever needed; the
    filter spectrum (tiny: 1 row) and all DFT/twiddle matrices are
    precomputed host-side and passed as constants.

Per row-pair dataflow (all 256x256 grids stored as (128,512) merged tiles,
block c = logical rows [128c,128c+128)):
  A[n1,n2] = z[n1*256+n2]            (n1>=128 is zero padding -> skipped)
  B^T[n2,k1] = sum_n1 A[n1,n2] F[n1,k1]         stage 1  (A-stationary)
  C^T = B^T * W_N^{k1 n2}                       fwd twiddle (DVE+POOL)
  D^T[k2,k1] = sum_n2 F[n2,k2] C^T[n2,k1]       stage 2  (F-stationary)
  P^T = D^T * S^T                               filter spectrum (DVE+POOL)
  E[k1,nL] = sum_k2 P^T[k2,k1] conjF[k2,nL]     stage 1' (P^T-stationary)
  G = E * W_N^{-nL k1} / N                      inv twiddle (DVE+POOL)
  Y[nH,nL] = sum_k1 conjF[k1,nH] G[k1,nL]       stage 2' (nH<128 only)
  y_a = Re(Y).flatten(), y_b = Im(Y).flatten()
"""

import os
from contextlib import ExitStack

import numpy as np

import concourse.bass as bass
from concourse import bacc
import concourse.mybir as mybir
import concourse.tile as tile
from concourse.bass_utils import run_bass_kernel_spmd

B, TLEN = 256, 32768
NFFT, R = 65536, 256
NCORES = 8
RPC = B // NCORES      # rows per core = 32
NPAIR = RPC // 2       # row-pairs per core = 16

F32 = mybir.dt.float32
ADD = mybir.AluOpType.add
SUB = mybir.AluOpType.subtract
MUL = mybir.AluOpType.mult

# matmul dtype: "f32" (exact) or "f32r" (fast fp32 mode, 4x matmul speed)
MM_DTYPE = os.environ.get("EFF_MM_DTYPE", "f32")

LAST_RESULT = None  # BassKernelResults of the most recent run (for test.py)

_PROGRAM_CACHE = {}


def _mrg(m):
    """(256,256) -> (128,512): [:, :256] = rows 0:128, [:, 256:] = rows 128:256."""
    return np.ascontiguousarray(
        np.concatenate([m[:128, :], m[128:, :]], axis=1), dtype=np.float32
    )


def _static_consts():
    k = np.arange(R, dtype=np.float64)
    ang_r = 2 * np.pi * np.outer(k, k) / R
    fr = np.cos(ang_r)
    fi = -np.sin(ang_r)            # F = exp(-2*pi*i*j*k/R)
    ang_n = 2 * np.pi * np.outer(k, k) / NFFT
    c = {
        "fr0": fr[:128, :], "fr1": fr[128:, :],
        "fi0": fi[:128, :], "fi1": fi[128:, :],
        "nfi0": -fi[:128, :], "nfi1": -fi[128:, :],
        "tfr": _mrg(np.cos(ang_n)),            # fwd twiddle [n2,k1]
        "tfi": _mrg(-np.sin(ang_n)),
        "tir": _mrg(np.cos(ang_n) / NFFT),     # inv twiddle [k1,nL], 1/N folded
        "tii": _mrg(np.sin(ang_n) / NFFT),
    }
    return {n: np.ascontiguousarray(v, dtype=np.float32) for n, v in c.items()}


def _filter_spectrum(filt):
    fpad = np.zeros(NFFT, dtype=np.float64)
    fpad[:TLEN] = filt.reshape(-1).astype(np.float64)
    s = np.fft.fft(fpad)
    st = s.reshape(R, R)           # [k2, k1] since S[k1 + 256*k2]
    return _mrg(st.real), _mrg(st.imag)


def _build_program(mm_dtype_name):
    nc = bacc.Bacc()
    x = nc.declare_dram_parameter("x", [RPC, TLEN], F32, isOutput=False)
    y = nc.declare_dram_parameter("y", [RPC, TLEN], F32, isOutput=True)
    small = ["fr0", "fr1", "fi0", "fi1", "nfi0", "nfi1"]
    big = ["tfr", "tfi", "tir", "tii", "sre", "sim"]
    dram = {n: nc.declare_dram_parameter(n, [128, 256], F32, isOutput=False)
            for n in small}
    dram.update({n: nc.declare_dram_parameter(n, [128, 512], F32, isOutput=False)
                 for n in big})

    if mm_dtype_name == "f32r":
        def mm_op(ap):
            return ap.bitcast(mybir.dt.float32r)
    else:
        def mm_op(ap):
            return ap

    xg = x.rearrange("b (p q) -> b p q", p=128)   # row -> (128,256) grid
    yg = y.rearrange("b (p q) -> b p q", p=128)

    with ExitStack() as ctx:
        tc = ctx.enter_context(tile.TileContext(nc))
        cpool = ctx.enter_context(tc.tile_pool(name="consts", bufs=1))
        C = {}
        for n in small:
            C[n] = cpool.tile([128, 256], F32, tag=n, name=n)
            nc.sync.dma_start(C[n][:], dram[n][:])
        for n in big:
            C[n] = cpool.tile([128, 512], F32, tag=n, name=n)
            nc.sync.dma_start(C[n][:], dram[n][:])

        apool = ctx.enter_context(tc.tile_pool(name="a", bufs=3))
        tpool = ctx.enter_context(tc.tile_pool(name="tmp", bufs=2))
        sbpool = ctx.enter_context(tc.tile_pool(name="sb", bufs=2))
        ypool = ctx.enter_context(tc.tile_pool(name="yout", bufs=3))
        pspool = ctx.enter_context(tc.tile_pool(name="ps", bufs=1, space="PSUM"))

        def cmul_evac(dst_re, dst_im, src_re, src_im, wre, wim, pref):
            """(dst_re + i dst_im) = (src_re + i src_im) * (wre + i wim).
            src in PSUM, w const in SBUF, dst in SBUF.
            DVE: 4 products (PSUM x SBUF), POOL: 2 combines (SBUF)."""
            t1 = tpool.tile([128, 512], F32, tag=f"{pref}_t1")
            t2 = tpool.tile([128, 512], F32, tag=f"{pref}_t2")
            t3 = tpool.tile([128, 512], F32, tag=f"{pref}_t3")
            t4 = tpool.tile([128, 512], F32, tag=f"{pref}_t4")
            nc.vector.tensor_tensor(t1[:], src_re[:], wre[:], MUL)
            nc.vector.tensor_tensor(t2[:], src_im[:], wim[:], MUL)
            nc.vector.tensor_tensor(t3[:], src_re[:], wim[:], MUL)
            nc.vector.tensor_tensor(t4[:], src_im[:], wre[:], MUL)
            nc.gpsimd.tensor_tensor(dst_re[:], t1[:], t2[:], SUB)
            nc.gpsimd.tensor_tensor(dst_im[:], t3[:], t4[:], ADD)

        for p in range(NPAIR):
            # ---- load row pair as complex grid A (only n1 < 128 nonzero)
            ar = apool.tile([128, 256], F32, tag="ar")
            ai = apool.tile([128, 256], F32, tag="ai")
            nc.sync.dma_start(ar[:], xg[2 * p])
            nc.sync.dma_start(ai[:], xg[2 * p + 1])

            # ---- stage 1: B^T[n2,k1] = sum_n1 A[n1,n2] F[n1,k1]
            btre = pspool.tile([128, 512], F32, tag="btre")
            btim = pspool.tile([128, 512], F32, tag="btim")
            for n2c in range(2):
                asl = slice(128 * n2c, 128 * n2c + 128)
                dre = btre[:, 256 * n2c:256 * n2c + 256]
                dim = btim[:, 256 * n2c:256 * n2c + 256]
                nc.tensor.matmul(dre, mm_op(ar[:, asl]), mm_op(C["fr0"][:]),
                                 start=True, stop=False)
                nc.tensor.matmul(dre, mm_op(ai[:, asl]), mm_op(C["nfi0"][:]),
                                 start=False, stop=True)
                nc.tensor.matmul(dim, mm_op(ar[:, asl]), mm_op(C["fi0"][:]),
                                 start=True, stop=False)
                nc.tensor.matmul(dim, mm_op(ai[:, asl]), mm_op(C["fr0"][:]),
                                 start=False, stop=True)

            # ---- fwd twiddle: C^T = B^T * W_N^{k1 n2}
            ctre = sbpool.tile([128, 512], F32, tag="ctre")
            ctim = sbpool.tile([128, 512], F32, tag="ctim")
            cmul_evac(ctre, ctim, btre, btim, C["tfr"], C["tfi"], "tw1")

            # ---- stage 2: D^T[k2,k1] = sum_n2 F[n2,k2] C^T[n2,k1]
            dtre = pspool.tile([128, 512], F32, tag="dtre")
            dtim = pspool.tile([128, 512], F32, tag="dtim")
            for k2c in range(2):
                fsl = slice(128 * k2c, 128 * k2c + 128)
                dre = dtre[:, 256 * k2c:256 * k2c + 256]
                dim = dtim[:, 256 * k2c:256 * k2c + 256]
                for n2c in range(2):
                    csl = slice(256 * n2c, 256 * n2c + 256)
                    frn = C[f"fr{n2c}"][:, fsl]
                    fin = C[f"fi{n2c}"][:, fsl]
                    nfin = C[f"nfi{n2c}"][:, fsl]
                    nc.tensor.matmul(dre, mm_op(frn), mm_op(ctre[:, csl]),
                                     start=(n2c == 0), stop=False)
                    nc.tensor.matmul(dre, mm_op(nfin), mm_op(ctim[:, csl]),
                                     start=False, stop=(n2c == 1))
                    nc.tensor.matmul(dim, mm_op(frn), mm_op(ctim[:, csl]),
                                     start=(n2c == 0), stop=False)
                    nc.tensor.matmul(dim, mm_op(fin), mm_op(ctre[:, csl]),
                                     start=False, stop=(n2c == 1))

            # ---- filter spectrum product: P^T = D^T * S^T
            ptre = sbpool.tile([128, 512], F32, tag="ptre")
            ptim = sbpool.tile([128, 512], F32, tag="ptim")
            cmul_evac(ptre, ptim, dtre, dtim, C["sre"], C["sim"], "sp")

            # ---- stage 1': E[k1,nL] = sum_k2 P^T[k2,k1] conjF[k2,nL]
            #      E_re = sum Pr*Fr + Pi*Fi ; E_im = sum -Pr*Fi + Pi*Fr
            ere = pspool.tile([128, 512], F32, tag="ere")
            eim = pspool.tile([128, 512], F32, tag="eim")
            for k1c in range(2):
                dre = ere[:, 256 * k1c:256 * k1c + 256]
                dim = eim[:, 256 * k1c:256 * k1c + 256]
                for k2c in range(2):
                    psl = slice(256 * k2c + 128 * k1c, 256 * k2c + 128 * k1c + 128)
                    nc.tensor.matmul(dre, mm_op(ptre[:, psl]), mm_op(C[f"fr{k2c}"][:]),
                                     start=(k2c == 0), stop=False)
                    nc.tensor.matmul(dre, mm_op(ptim[:, psl]), mm_op(C[f"fi{k2c}"][:]),
                                     start=False, stop=(k2c == 1))
                    nc.tensor.matmul(dim, mm_op(ptre[:, psl]), mm_op(C[f"nfi{k2c}"][:]),
                                     start=(k2c == 0), stop=False)
                    nc.tensor.matmul(dim, mm_op(ptim[:, psl]), mm_op(C[f"fr{k2c}"][:]),
                                     start=False, stop=(k2c == 1))

            # ---- inv twiddle: G = E * W_N^{-nL k1} / N
            gre = sbpool.tile([128, 512], F32, tag="gre")
            gim = sbpool.tile([128, 512], F32, tag="gim")
            cmul_evac(gre, gim, ere, eim, C["tir"], C["tii"], "tw2")

            # ---- stage 2': Y[nH,nL] = sum_k1 conjF[k1,nH] G[k1,nL], nH<128
            #      Y_re = sum Fr.T Gr + Fi.T Gi ; Y_im = sum Fr.T Gi - Fi.T Gr
            # (yre and yim share one PSUM bank: the two accumulation groups
            # must be sequential, not interleaved)
            yps = pspool.tile([128, 512], F32, tag="yps")
            yre = yps[:, 0:256]
            yim = yps[:, 256:512]
            for k1c in range(2):
                gsl = slice(256 * k1c, 256 * k1c + 256)
                frh = C[f"fr{k1c}"][:, 0:128]
                fih = C[f"fi{k1c}"][:, 0:128]
                nc.tensor.matmul(yre, mm_op(frh), mm_op(gre[:, gsl]),
                                 start=(k1c == 0), stop=False)
                nc.tensor.matmul(yre, mm_op(fih), mm_op(gim[:, gsl]),
                                 start=False, stop=(k1c == 1))
            for k1c in range(2):
                gsl = slice(256 * k1c, 256 * k1c + 256)
                frh = C[f"fr{k1c}"][:, 0:128]
                nfih = C[f"nfi{k1c}"][:, 0:128]
                nc.tensor.matmul(yim, mm_op(frh), mm_op(gim[:, gsl]),
                                 start=(k1c == 0), stop=False)
                nc.tensor.matmul(yim, mm_op(nfih), mm_op(gre[:, gsl]),
                                 start=False, stop=(k1c == 1))

            # ---- evacuate + store
            ysb = ypool.tile([128, 512], F32, tag="ysb")
            nc.scalar.copy(ysb[:], yps[:])
            nc.sync.dma_start(yg[2 * p], ysb[:, 0:256])
            nc.sync.dma_start(yg[2 * p + 1], ysb[:, 256:512])

    nc.compile()
    return nc


def _get_program(mm_dtype_name):
    if mm_dtype_name not in _PROGRAM_CACHE:
        _PROGRAM_CACHE[mm_dtype_name] = _build_program(mm_dtype_name)
    return _PROGRAM_CACHE[mm_dtype_name]


def kernel(x, filt):
    global LAST_RESULT
    assert x.shape == (B, TLEN) and x.dtype == np.float32
    nc = _get_program(MM_DTYPE)

    consts = dict(_static_consts())
    consts["sre"], consts["sim"] = _filter_spectrum(filt)

    in_maps = []
    for c in range(NCORES):
        m = dict(consts)
        m["x"] = np.ascontiguousarray(x[c * RPC:(c + 1) * RPC])
        in_maps.append(m)

    res = run_bass_kernel_spmd(
        nc, in_maps, core_ids=list(range(NCORES)),
        trace=bool(int(os.environ.get("EFF_TRACE", "0"))),
    )
    LAST_RESULT = res
    return np.concatenate([r["y"] for r in res.results], axis=0)


# revision 6
# speedup vs baseline: 2.2643x; 2.2643x over previous
"""Trainium2 Bass kernel for EpochedFutureFill: exact causal convolution
y[b,t] = sum_s filt[s] x[b,t-s], computed via a 65536-point FFT per row.

Strategy (data-parallel over 8 cores, 32 rows each):
  - Pack two real rows per complex signal: z = x_a + i*x_b. Since
    IFFT(S * FFT(z)) is linear over R^2, Re -> conv(x_a), Im -> conv(x_b).
  - 65536-pt FFT as 2-stage Cooley-Tukey (256 x 256) built from 256-pt DFT
    matmuls on the tensor engine; twiddle/spectrum pointwise multiplies on
    the vector engine (+ gpsimd for SBUF-side combines).
  - Matmul orientations chosen so no transposes are ever needed; the
    filter spectrum (tiny: 1 row) and all DFT/twiddle matrices are
    precomputed host-side and passed as constants.

Per row-pair dataflow (all 256x256 grids stored as (128,512) merged tiles,
block c = logical rows [128c,128c+128)):
  A[n1,n2] = z[n1*256+n2]            (n1>=128 is zero padding -> skipped)
  B^T[n2,k1] = sum_n1 A[n1,n2] F[n1,k1]         stage 1  (A-stationary)
  C^T = B^T * W_N^{k1 n2}                       fwd twiddle (DVE+POOL)
  D^T[k2,k1] = sum_n2 F[n2,k2] C^T[n2,k1]       stage 2  (F-stationary)
  P^T = D^T * S^T                               filter spectrum (DVE+POOL)
  E[k1,nL] = sum_k2 P^T[k2,k1] conjF[k2,nL]     stage 1' (P^T-stationary)
  G = E * W_N^{-nL k1} / N                      inv twiddle (DVE+POOL)
  Y[nH,nL] = sum_k1 conjF[k1,nH] G[k1,nL]       stage 2' (nH<128 only)
  y_a = Re(Y).flatten(), y_b = Im(Y).flatten()
"""

import os
from contextlib import ExitStack

import numpy as np

import concourse.bass as bass
from concourse import bacc
import concourse.mybir as mybir
import concourse.tile as tile
from concourse.bass_utils import run_bass_kernel_spmd

B, TLEN = 256, 32768
NFFT, R = 65536, 256
NCORES = 8
RPC = B // NCORES      # rows per core = 32
NPAIR = RPC // 2       # row-pairs per core = 16

F32 = mybir.dt.float32
ADD = mybir.AluOpType.add
SUB = mybir.AluOpType.subtract
MUL = mybir.AluOpType.mult

# matmul dtype: "f32" (exact) or "f32r" (fast fp32 mode, 4x matmul speed)
MM_DTYPE = os.environ.get("EFF_MM_DTYPE", "f32")

LAST_RESULT = None  # BassKernelResults of the most recent run (for test.py)

_PROGRAM_CACHE = {}


def _mrg(m):
    """(256,256) -> (128,512): [:, :256] = rows 0:128, [:, 256:] = rows 128:256."""
    return np.ascontiguousarray(
        np.concatenate([m[:128, :], m[128:, :]], axis=1), dtype=np.float32
    )


def _static_consts():
    k = np.arange(R, dtype=np.float64)
    ang_r = 2 * np.pi * np.outer(k, k) / R
    fr = np.cos(ang_r)
    fi = -np.sin(ang_r)            # F = exp(-2*pi*i*j*k/R)
    ang_n = 2 * np.pi * np.outer(k, k) / NFFT
    c = {
        "fr0": fr[:128, :], "fr1": fr[128:, :],
        "fi0": fi[:128, :], "fi1": fi[128:, :],
        "nfi0": -fi[:128, :], "nfi1": -fi[128:, :],
        "tfr": _mrg(np.cos(ang_n)),            # fwd twiddle [n2,k1]
        "tfi": _mrg(-np.sin(ang_n)),
        "tir": _mrg(np.cos(ang_n) / NFFT),     # inv twiddle [k1,nL], 1/N folded
        "tii": _mrg(np.sin(ang_n) / NFFT),
    }
    return {n: np.ascontiguousarray(v, dtype=np.float32) for n, v in c.items()}


def _filter_spectrum(filt):
    fpad = np.zeros(NFFT, dtype=np.float64)
    fpad[:TLEN] = filt.reshape(-1).astype(np.float64)
    s = np.fft.fft(fpad)
    st = s.reshape(R, R)           # [k2, k1] since S[k1 + 256*k2]
    return _mrg(st.real), _mrg(st.imag)


def _build_program(mm_dtype_name):
    nc = bacc.Bacc()
    MMDT0 = mybir.dt.float32r if mm_dtype_name == "f32r" else F32
    x = nc.declare_dram_parameter("x", [RPC, TLEN], MMDT0, isOutput=False)
    y = nc.declare_dram_parameter("y", [RPC, TLEN], F32, isOutput=True)
    small = ["fr0", "fr1", "fi0", "fi1", "nfi0", "nfi1"]
    big = ["tfr", "tfi", "tir", "tii", "sre", "sim"]
    dram = {n: nc.declare_dram_parameter(n, [128, 256], MMDT0, isOutput=False)
            for n in small}
    dram.update({n: nc.declare_dram_parameter(n, [128, 512], F32, isOutput=False)
                 for n in big})

    MMDT = mybir.dt.float32r if mm_dtype_name == "f32r" else F32

    def mm_op(ap):
        return ap

    xg = x.rearrange("b (p q) -> b p q", p=128)   # row -> (128,256) grid
    yg = y.rearrange("b (p q) -> b p q", p=128)

    with ExitStack() as ctx:
        tc = ctx.enter_context(tile.TileContext(nc))
        cpool = ctx.enter_context(tc.tile_pool(name="consts", bufs=1))
        C = {}
        for n in small:
            C[n] = cpool.tile([128, 256], MMDT, tag=n, name=n)
            nc.sync.dma_start(C[n][:], dram[n][:])
        for n in big:
            C[n] = cpool.tile([128, 512], F32, tag=n, name=n)
            nc.sync.dma_start(C[n][:], dram[n][:])

        apool = ctx.enter_context(tc.tile_pool(name="a", bufs=3))
        tpool = ctx.enter_context(tc.tile_pool(name="tmp", bufs=2))
        sbpool = ctx.enter_context(tc.tile_pool(name="sb", bufs=2))
        ypool = ctx.enter_context(tc.tile_pool(name="yout", bufs=3))
        pspool = ctx.enter_context(tc.tile_pool(name="ps", bufs=1, space="PSUM"))

        def cmul_evac(dst_re, dst_im, src_re, src_im, wre, wim, pref):
            """(dst_re + i dst_im) = (src_re + i src_im) * (wre + i wim).
            src in PSUM, w const in SBUF, dst in SBUF.
            DVE: 4 products (PSUM x SBUF), POOL: 2 combines (SBUF)."""
            t1 = tpool.tile([128, 512], F32, tag=f"{pref}_t1")
            t2 = tpool.tile([128, 512], F32, tag=f"{pref}_t2")
            t3 = tpool.tile([128, 512], F32, tag=f"{pref}_t3")
            t4 = tpool.tile([128, 512], F32, tag=f"{pref}_t4")
            nc.vector.tensor_tensor(t1[:], src_re[:], wre[:], MUL)
            nc.vector.tensor_tensor(t2[:], src_im[:], wim[:], MUL)
            nc.vector.tensor_tensor(t3[:], src_re[:], wim[:], MUL)
            nc.vector.tensor_tensor(t4[:], src_im[:], wre[:], MUL)
            nc.gpsimd.tensor_tensor(dst_re[:], t1[:], t2[:], SUB)
            nc.gpsimd.tensor_tensor(dst_im[:], t3[:], t4[:], ADD)

        for p in range(NPAIR):
            # ---- load row pair as complex grid A (only n1 < 128 nonzero)
            ar = apool.tile([128, 256], MMDT, tag="ar")
            ai = apool.tile([128, 256], MMDT, tag="ai")
            nc.sync.dma_start(ar[:], xg[2 * p])
            nc.sync.dma_start(ai[:], xg[2 * p + 1])

            # ---- stage 1: B^T[n2,k1] = sum_n1 A[n1,n2] F[n1,k1]
            btre = pspool.tile([128, 512], F32, tag="btre")
            btim = pspool.tile([128, 512], F32, tag="btim")
            for n2c in range(2):
                asl = slice(128 * n2c, 128 * n2c + 128)
                dre = btre[:, 256 * n2c:256 * n2c + 256]
                dim = btim[:, 256 * n2c:256 * n2c + 256]
                nc.tensor.matmul(dre, mm_op(ar[:, asl]), mm_op(C["fr0"][:]),
                                 start=True, stop=False)
                nc.tensor.matmul(dre, mm_op(ai[:, asl]), mm_op(C["nfi0"][:]),
                                 start=False, stop=True)
                nc.tensor.matmul(dim, mm_op(ar[:, asl]), mm_op(C["fi0"][:]),
                                 start=True, stop=False)
                nc.tensor.matmul(dim, mm_op(ai[:, asl]), mm_op(C["fr0"][:]),
                                 start=False, stop=True)

            # ---- fwd twiddle: C^T = B^T * W_N^{k1 n2}
            ctre = sbpool.tile([128, 512], MMDT, tag="ctre")
            ctim = sbpool.tile([128, 512], MMDT, tag="ctim")
            cmul_evac(ctre, ctim, btre, btim, C["tfr"], C["tfi"], "tw1")

            # ---- stage 2: D^T[k2,k1] = sum_n2 F[n2,k2] C^T[n2,k1]
            dtre = pspool.tile([128, 512], F32, tag="dtre")
            dtim = pspool.tile([128, 512], F32, tag="dtim")
            for k2c in range(2):
                fsl = slice(128 * k2c, 128 * k2c + 128)
                dre = dtre[:, 256 * k2c:256 * k2c + 256]
                dim = dtim[:, 256 * k2c:256 * k2c + 256]
                for n2c in range(2):
                    csl = slice(256 * n2c, 256 * n2c + 256)
                    frn = C[f"fr{n2c}"][:, fsl]
                    fin = C[f"fi{n2c}"][:, fsl]
                    nfin = C[f"nfi{n2c}"][:, fsl]
                    nc.tensor.matmul(dre, mm_op(frn), mm_op(ctre[:, csl]),
                                     start=(n2c == 0), stop=False)
                    nc.tensor.matmul(dre, mm_op(nfin), mm_op(ctim[:, csl]),
                                     start=False, stop=(n2c == 1))
                    nc.tensor.matmul(dim, mm_op(frn), mm_op(ctim[:, csl]),
                                     start=(n2c == 0), stop=False)
                    nc.tensor.matmul(dim, mm_op(fin), mm_op(ctre[:, csl]),
                                     start=False, stop=(n2c == 1))

            # ---- filter spectrum product: P^T = D^T * S^T
            ptre = sbpool.tile([128, 512], MMDT, tag="ptre")
            ptim = sbpool.tile([128, 512], MMDT, tag="ptim")
            cmul_evac(ptre, ptim, dtre, dtim, C["sre"], C["sim"], "sp")

            # ---- stage 1': E[k1,nL] = sum_k2 P^T[k2,k1] conjF[k2,nL]
            #      E_re = sum Pr*Fr + Pi*Fi ; E_im = sum -Pr*Fi + Pi*Fr
            ere = pspool.tile([128, 512], F32, tag="ere")
            eim = pspool.tile([128, 512], F32, tag="eim")
            for k1c in range(2):
                dre = ere[:, 256 * k1c:256 * k1c + 256]
                dim = eim[:, 256 * k1c:256 * k1c + 256]
                for k2c in range(2):
                    psl = slice(256 * k2c + 128 * k1c, 256 * k2c + 128 * k1c + 128)
                    nc.tensor.matmul(dre, mm_op(ptre[:, psl]), mm_op(C[f"fr{k2c}"][:]),
                                     start=(k2c == 0), stop=False)
                    nc.tensor.matmul(dre, mm_op(ptim[:, psl]), mm_op(C[f"fi{k2c}"][:]),
                                     start=False, stop=(k2c == 1))
                    nc.tensor.matmul(dim, mm_op(ptre[:, psl]), mm_op(C[f"nfi{k2c}"][:]),
                                     start=(k2c == 0), stop=False)
                    nc.tensor.matmul(dim, mm_op(ptim[:, psl]), mm_op(C[f"fr{k2c}"][:]),
                                     start=False, stop=(k2c == 1))

            # ---- inv twiddle: G = E * W_N^{-nL k1} / N
            gre = sbpool.tile([128, 512], MMDT, tag="gre")
            gim = sbpool.tile([128, 512], MMDT, tag="gim")
            cmul_evac(gre, gim, ere, eim, C["tir"], C["tii"], "tw2")

            # ---- stage 2': Y[nH,nL] = sum_k1 conjF[k1,nH] G[k1,nL], nH<128
            #      Y_re = sum Fr.T Gr + Fi.T Gi ; Y_im = sum Fr.T Gi - Fi.T Gr
            # (yre and yim share one PSUM bank: the two accumulation groups
            # must be sequential, not interleaved)
            yps = pspool.tile([128, 512], F32, tag="yps")
            yre = yps[:, 0:256]
            yim = yps[:, 256:512]
            for k1c in range(2):
                gsl = slice(256 * k1c, 256 * k1c + 256)
                frh = C[f"fr{k1c}"][:, 0:128]
                fih = C[f"fi{k1c}"][:, 0:128]
                nc.tensor.matmul(yre, mm_op(frh), mm_op(gre[:, gsl]),
                                 start=(k1c == 0), stop=False)
                nc.tensor.matmul(yre, mm_op(fih), mm_op(gim[:, gsl]),
                                 start=False, stop=(k1c == 1))
            for k1c in range(2):
                gsl = slice(256 * k1c, 256 * k1c + 256)
                frh = C[f"fr{k1c}"][:, 0:128]
                nfih = C[f"nfi{k1c}"][:, 0:128]
                nc.tensor.matmul(yim, mm_op(frh), mm_op(gim[:, gsl]),
                                 start=(k1c == 0), stop=False)
                nc.tensor.matmul(yim, mm_op(nfih), mm_op(gre[:, gsl]),
                                 start=False, stop=(k1c == 1))

            # ---- evacuate + store
            ysb = ypool.tile([128, 512], F32, tag="ysb")
            nc.scalar.copy(ysb[:], yps[:])
            nc.sync.dma_start(yg[2 * p], ysb[:, 0:256])
            nc.sync.dma_start(yg[2 * p + 1], ysb[:, 256:512])

    nc.compile()
    return nc


def _get_program(mm_dtype_name):
    if mm_dtype_name not in _PROGRAM_CACHE:
        _PROGRAM_CACHE[mm_dtype_name] = _build_program(mm_dtype_name)
    return _PROGRAM_CACHE[mm_dtype_name]


def kernel(x, filt):
    global LAST_RESULT
    assert x.shape == (B, TLEN) and x.dtype == np.float32
    nc = _get_program(MM_DTYPE)

    consts = dict(_static_consts())
    consts["sre"], consts["sim"] = _filter_spectrum(filt)

    in_maps = []
    for c in range(NCORES):
        m = dict(consts)
        m["x"] = np.ascontiguousarray(x[c * RPC:(c + 1) * RPC])
        in_maps.append(m)

    res = run_bass_kernel_spmd(
        nc, in_maps, core_ids=list(range(NCORES)),
        trace=bool(int(os.environ.get("EFF_TRACE", "0"))),
    )
    LAST_RESULT = res
    return np.concatenate([r["y"] for r in res.results], axis=0)


# revision 7
# speedup vs baseline: 2.2729x; 1.0038x over previous
"""Trainium2 Bass kernel for EpochedFutureFill: exact causal convolution
y[b,t] = sum_s filt[s] x[b,t-s], computed via a 65536-point FFT per row.

Strategy (data-parallel over 8 cores, 32 rows each):
  - Pack two real rows per complex signal: z = x_a + i*x_b. Since
    IFFT(S * FFT(z)) is linear over R^2, Re -> conv(x_a), Im -> conv(x_b).
  - 65536-pt FFT as 2-stage Cooley-Tukey (256 x 256) built from 256-pt DFT
    matmuls on the tensor engine (float32r mode: full-rate fp32 matmul);
    twiddle/spectrum pointwise multiplies on the vector engine with wide
    broadcast reads + gpsimd for the SBUF-side combines.
  - Matmul orientations chosen so no transposes are ever needed; the
    filter spectrum (tiny: 1 row) and all DFT/twiddle matrices are
    precomputed host-side and passed as constants.

Per row-pair dataflow (256x256 grids stored as (128,512) merged tiles,
block c = logical rows [128c,128c+128); complex SBUF intermediates are
(128,1024) = [re(512) | im(512)]):
  A[n1,n2] = z[n1*256+n2]            (n1>=128 is zero padding -> skipped)
  B^T[n2,k1] = sum_n1 A[n1,n2] F[n1,k1]         stage 1  (A-stationary)
  C^T = B^T * W_N^{k1 n2}                       fwd twiddle (DVE+POOL)
  D^T[k2,k1] = sum_n2 F[n2,k2] C^T[n2,k1]       stage 2  (F-stationary)
  P^T = D^T * S^T                               filter spectrum (DVE+POOL)
  E[k1,nL] = sum_k2 P^T[k2,k1] conjF[k2,nL]     stage 1\' (P^T-stationary)
  G = E * W_N^{-nL k1} / N                      inv twiddle (DVE+POOL)
  Y[nH,nL] = sum_k1 conjF[k1,nH] G[k1,nL]       stage 2\' (nH<128 only)
  y_a = Re(Y).flatten(), y_b = Im(Y).flatten()

Each pointwise complex multiply z*w is 3 wide ops:
  ta = [zr*wr | zr*wi]   (DVE, broadcast-read zr from PSUM)
  tb = [-zi*wi | zi*wr]  (DVE, broadcast-read zi from PSUM)
  dst = ta + tb = [re|im]  (GpSimd, both halves ADD)
"""

import os
from contextlib import ExitStack

import numpy as np

import concourse.bass as bass
import concourse.mybir as mybir
import concourse.tile as tile
from concourse import bacc
from concourse.bass_utils import run_bass_kernel_spmd

B, TLEN = 256, 32768
NFFT, R = 65536, 256
NCORES = 8
RPC = B // NCORES      # rows per core = 32
NPAIR = RPC // 2       # row-pairs per core = 16

F32 = mybir.dt.float32
ADD = mybir.AluOpType.add
SUB = mybir.AluOpType.subtract
MUL = mybir.AluOpType.mult

# matmul dtype: "f32" (exact, 4 cyc/row) or "f32r" (1 cyc/row, ~2.5e-4 rel err)
MM_DTYPE = os.environ.get("EFF_MM_DTYPE", "f32r")

LAST_RESULT = None  # BassKernelResults of the most recent run (for test.py)

_PROGRAM_CACHE = {}


def _mrg(m):
    """(256,256) -> (128,512): [:, :256] = rows 0:128, [:, 256:] = rows 128:256."""
    return np.concatenate([m[:128, :], m[128:, :]], axis=1)


def _wpair(wr, wi):
    """Combined all-ADD grids: wa=[wr|wi], wb=[-wi|wr], each (128,1024) f32."""
    wa = np.concatenate([_mrg(wr), _mrg(wi)], axis=1)
    wb = np.concatenate([_mrg(-wi), _mrg(wr)], axis=1)
    return (np.ascontiguousarray(wa, dtype=np.float32),
            np.ascontiguousarray(wb, dtype=np.float32))


def _static_consts():
    k = np.arange(R, dtype=np.float64)
    ang_r = 2 * np.pi * np.outer(k, k) / R
    fr = np.cos(ang_r)
    fi = -np.sin(ang_r)            # F = exp(-2*pi*i*j*k/R)
    ang_n = 2 * np.pi * np.outer(k, k) / NFFT
    c = {
        "fr0": fr[:128, :], "fr1": fr[128:, :],
        "fi0": fi[:128, :], "fi1": fi[128:, :],
        "nfi0": -fi[:128, :], "nfi1": -fi[128:, :],
    }
    c = {n: np.ascontiguousarray(v, dtype=np.float32) for n, v in c.items()}
    c["tfa"], c["tfb"] = _wpair(np.cos(ang_n), -np.sin(ang_n))          # [n2,k1]
    c["tia"], c["tib"] = _wpair(np.cos(ang_n) / NFFT, np.sin(ang_n) / NFFT)
    return c


def _filter_spectrum(filt):
    fpad = np.zeros(NFFT, dtype=np.float64)
    fpad[:TLEN] = filt.reshape(-1).astype(np.float64)
    s = np.fft.fft(fpad)
    st = s.reshape(R, R)           # [k2, k1] since S[k1 + 256*k2]
    return _wpair(st.real, st.imag)


def _build_program(mm_dtype_name):
    nc = bacc.Bacc()
    MMDT = mybir.dt.float32r if mm_dtype_name == "f32r" else F32
    x = nc.declare_dram_parameter("x", [RPC, TLEN], MMDT, isOutput=False)
    y = nc.declare_dram_parameter("y", [RPC, TLEN], F32, isOutput=True)
    small = ["fr0", "fr1", "fi0", "fi1", "nfi0", "nfi1"]
    big = ["tfa", "tfb", "spa", "spb", "tia", "tib"]
    dram = {n: nc.declare_dram_parameter(n, [128, 256], MMDT, isOutput=False)
            for n in small}
    dram.update({n: nc.declare_dram_parameter(n, [128, 1024], F32, isOutput=False)
                 for n in big})

    xg = x.rearrange("b (p q) -> b p q", p=128)   # row -> (128,256) grid
    yg = y.rearrange("b (p q) -> b p q", p=128)

    with ExitStack() as ctx:
        tc = ctx.enter_context(tile.TileContext(nc))
        cpool = ctx.enter_context(tc.tile_pool(name="consts", bufs=1))
        C = {}
        # order: earliest-needed consts first so rp0 can start ASAP
        order = [("fr0", 256), ("fi0", 256), ("nfi0", 256),
                 ("tfa", 1024), ("tfb", 1024),
                 ("fr1", 256), ("fi1", 256), ("nfi1", 256),
                 ("spa", 1024), ("spb", 1024), ("tia", 1024), ("tib", 1024)]
        for n, w in order:
            dt_ = MMDT if w == 256 else F32
            C[n] = cpool.tile([128, w], dt_, tag=n, name=n)
            nc.sync.dma_start(C[n][:], dram[n][:])

        apool = ctx.enter_context(tc.tile_pool(name="a", bufs=4))
        tpool = ctx.enter_context(tc.tile_pool(name="tmp", bufs=2))
        sbpool = ctx.enter_context(tc.tile_pool(name="sb", bufs=2))
        ypool = ctx.enter_context(tc.tile_pool(name="yout", bufs=3))
        pspool = ctx.enter_context(tc.tile_pool(name="ps", bufs=1, space="PSUM"))

        def v3(t):
            return t.rearrange("p (a q) -> p a q", a=2)

        def cmul_evac(dst, src_re, src_im, wa, wb, pref):
            """dst (128,1024)=[re|im] = (src_re + i src_im) * w, all-ADD form.
            src_* in PSUM (128,512); wa/wb (128,1024) const."""
            ta = tpool.tile([128, 1024], F32, tag=f"{pref}_ta")
            tb = tpool.tile([128, 1024], F32, tag=f"{pref}_tb")
            sr = src_re[:, None, :].to_broadcast([128, 2, 512])
            si = src_im[:, None, :].to_broadcast([128, 2, 512])
            nc.vector.tensor_tensor(v3(ta[:]), sr, v3(wa[:]), MUL)
            nc.vector.tensor_tensor(v3(tb[:]), si, v3(wb[:]), MUL)
            nc.gpsimd.tensor_tensor(dst[:], ta[:], tb[:], ADD)

        for p in range(NPAIR):
            # ---- load row pair as complex grid A (only n1 < 128 nonzero)
            ar = apool.tile([128, 256], MMDT, tag="ar")
            ai = apool.tile([128, 256], MMDT, tag="ai")
            nc.sync.dma_start(ar[:], xg[2 * p])
            nc.sync.dma_start(ai[:], xg[2 * p + 1])

            # ---- stage 1: B^T[n2,k1] = sum_n1 A[n1,n2] F[n1,k1]
            btre = pspool.tile([128, 512], F32, tag="btre")
            btim = pspool.tile([128, 512], F32, tag="btim")
            for n2c in range(2):
                asl = slice(128 * n2c, 128 * n2c + 128)
                dre = btre[:, 256 * n2c:256 * n2c + 256]
                dim = btim[:, 256 * n2c:256 * n2c + 256]
                nc.tensor.matmul(dre, ar[:, asl], C["fr0"][:],
                                 start=True, stop=False)
                nc.tensor.matmul(dre, ai[:, asl], C["nfi0"][:],
                                 start=False, stop=True)
                nc.tensor.matmul(dim, ar[:, asl], C["fi0"][:],
                                 start=True, stop=False)
                nc.tensor.matmul(dim, ai[:, asl], C["fr0"][:],
                                 start=False, stop=True)

            # ---- fwd twiddle: C^T = B^T * W_N^{k1 n2}
            ct = sbpool.tile([128, 1024], MMDT, tag="ct")
            cmul_evac(ct, btre, btim, C["tfa"], C["tfb"], "tw1")
            ctre = ct[:, 0:512]
            ctim = ct[:, 512:1024]

            # ---- stage 2: D^T[k2,k1] = sum_n2 F[n2,k2] C^T[n2,k1]
            dtre = pspool.tile([128, 512], F32, tag="dtre")
            dtim = pspool.tile([128, 512], F32, tag="dtim")
            for k2c in range(2):
                fsl = slice(128 * k2c, 128 * k2c + 128)
                dre = dtre[:, 256 * k2c:256 * k2c + 256]
                dim = dtim[:, 256 * k2c:256 * k2c + 256]
                for n2c in range(2):
                    csl = slice(256 * n2c, 256 * n2c + 256)
                    frn = C[f"fr{n2c}"][:, fsl]
                    fin = C[f"fi{n2c}"][:, fsl]
                    nfin = C[f"nfi{n2c}"][:, fsl]
                    nc.tensor.matmul(dre, frn, ctre[:, csl],
                                     start=(n2c == 0), stop=False)
                    nc.tensor.matmul(dre, nfin, ctim[:, csl],
                                     start=False, stop=(n2c == 1))
                    nc.tensor.matmul(dim, frn, ctim[:, csl],
                                     start=(n2c == 0), stop=False)
                    nc.tensor.matmul(dim, fin, ctre[:, csl],
                                     start=False, stop=(n2c == 1))

            # ---- filter spectrum product: P^T = D^T * S^T
            pt = sbpool.tile([128, 1024], MMDT, tag="pt")
            cmul_evac(pt, dtre, dtim, C["spa"], C["spb"], "sp")
            ptre = pt[:, 0:512]
            ptim = pt[:, 512:1024]

            # ---- stage 1\': E[k1,nL] = sum_k2 P^T[k2,k1] conjF[k2,nL]
            #      E_re = sum Pr*Fr + Pi*Fi ; E_im = sum -Pr*Fi + Pi*Fr
            ere = pspool.tile([128, 512], F32, tag="ere")
            eim = pspool.tile([128, 512], F32, tag="eim")
            for k1c in range(2):
                dre = ere[:, 256 * k1c:256 * k1c + 256]
                dim = eim[:, 256 * k1c:256 * k1c + 256]
                for k2c in range(2):
                    psl = slice(256 * k2c + 128 * k1c, 256 * k2c + 128 * k1c + 128)
                    nc.tensor.matmul(dre, ptre[:, psl], C[f"fr{k2c}"][:],
                                     start=(k2c == 0), stop=False)
                    nc.tensor.matmul(dre, ptim[:, psl], C[f"fi{k2c}"][:],
                                     start=False, stop=(k2c == 1))
                    nc.tensor.matmul(dim, ptre[:, psl], C[f"nfi{k2c}"][:],
                                     start=(k2c == 0), stop=False)
                    nc.tensor.matmul(dim, ptim[:, psl], C[f"fr{k2c}"][:],
                                     start=False, stop=(k2c == 1))

            # ---- inv twiddle: G = E * W_N^{-nL k1} / N
            g = sbpool.tile([128, 1024], MMDT, tag="g")
            cmul_evac(g, ere, eim, C["tia"], C["tib"], "tw2")
            gre = g[:, 0:512]
            gim = g[:, 512:1024]

            # ---- stage 2\': Y[nH,nL] = sum_k1 conjF[k1,nH] G[k1,nL], nH<128
            #      Y_re = sum Fr.T Gr + Fi.T Gi ; Y_im = sum Fr.T Gi - Fi.T Gr
            # (yre and yim share one PSUM bank: the two accumulation groups
            # must be sequential, not interleaved)
            yps = pspool.tile([128, 512], F32, tag="yps")
            yre = yps[:, 0:256]
            yim = yps[:, 256:512]
            for k1c in range(2):
                gsl = slice(256 * k1c, 256 * k1c + 256)
                frh = C[f"fr{k1c}"][:, 0:128]
                fih = C[f"fi{k1c}"][:, 0:128]
                nc.tensor.matmul(yre, frh, gre[:, gsl],
                                 start=(k1c == 0), stop=False)
                nc.tensor.matmul(yre, fih, gim[:, gsl],
                                 start=False, stop=(k1c == 1))
            for k1c in range(2):
                gsl = slice(256 * k1c, 256 * k1c + 256)
                frh = C[f"fr{k1c}"][:, 0:128]
                nfih = C[f"nfi{k1c}"][:, 0:128]
                nc.tensor.matmul(yim, frh, gim[:, gsl],
                                 start=(k1c == 0), stop=False)
                nc.tensor.matmul(yim, nfih, gre[:, gsl],
                                 start=False, stop=(k1c == 1))

            # ---- evacuate + store
            ysb = ypool.tile([128, 512], F32, tag="ysb")
            nc.scalar.copy(ysb[:], yps[:])
            nc.sync.dma_start(yg[2 * p], ysb[:, 0:256])
            nc.sync.dma_start(yg[2 * p + 1], ysb[:, 256:512])

    nc.compile()
    return nc


def _get_program(mm_dtype_name):
    if mm_dtype_name not in _PROGRAM_CACHE:
        _PROGRAM_CACHE[mm_dtype_name] = _build_program(mm_dtype_name)
    return _PROGRAM_CACHE[mm_dtype_name]


def kernel(x, filt):
    global LAST_RESULT
    assert x.shape == (B, TLEN) and x.dtype == np.float32
    nc = _get_program(MM_DTYPE)

    consts = dict(_static_consts())
    consts["spa"], consts["spb"] = _filter_spectrum(filt)

    in_maps = []
    for c in range(NCORES):
        m = dict(consts)
        m["x"] = np.ascontiguousarray(x[c * RPC:(c + 1) * RPC])
        in_maps.append(m)

    res = run_bass_kernel_spmd(
        nc, in_maps, core_ids=list(range(NCORES)),
        trace=bool(int(os.environ.get("EFF_TRACE", "0"))),
    )
    LAST_RESULT = res
    return np.concatenate([r["y"] for r in res.results], axis=0)


# revision 8
# speedup vs baseline: 2.3284x; 1.0244x over previous
"""Trainium2 Bass kernel for EpochedFutureFill: exact causal convolution
y[b,t] = sum_s filt[s] x[b,t-s], computed via a 65536-point FFT per row.

Strategy (data-parallel over 8 cores, 32 rows each):
  - Pack two real rows per complex signal: z = x_a + i*x_b. Since
    IFFT(S * FFT(z)) is linear over R^2, Re -> conv(x_a), Im -> conv(x_b).
  - 65536-pt FFT as 2-stage Cooley-Tukey (256 x 256) built from 256-pt DFT
    matmuls on the tensor engine (float32r mode: full-rate fp32 matmul);
    twiddle/spectrum pointwise multiplies on the vector engine with
    broadcast reads + gpsimd for the SBUF-side combines.
  - Matmul orientations chosen so no transposes are ever needed; the
    filter spectrum (tiny: 1 row) and all DFT/twiddle matrices are
    precomputed host-side and passed as constants.
  - Row-pairs are software-pipelined: emission order skews the second half
    of pair p-1 between the two halves of pair p, so each engine always
    has independent ready work.

Per row-pair dataflow (256x256 grids; complex SBUF intermediates are
(128,1024) chunk-major: [re_c0(256)|im_c0|re_c1|im_c1]):
  A[n1,n2] = z[n1*256+n2]            (n1>=128 is zero padding -> skipped)
  S1: B^T[n2,k1] = sum_n1 A[n1,n2] F[n1,k1]      (A-stationary)
  tw1: C^T = B^T * W_N^{k1 n2}                   (DVE+POOL)
  S2: D^T[k2,k1] = sum_n2 F[n2,k2] C^T[n2,k1]    (F-stationary)
  sp: P^T = D^T * S^T                            (DVE+POOL)
  S3: E[k1,nL] = sum_k2 P^T[k2,k1] conjF[k2,nL]  (P^T-stationary)
  tw2: G = E * W_N^{-nL k1} / N                  (DVE+POOL)
  S4: Y[nH,nL] = sum_k1 conjF[k1,nH] G[k1,nL]    (nH<128 only)
  y_a = Re(Y).flatten(), y_b = Im(Y).flatten()

Each pointwise complex multiply z*w is per-chunk (c = 0,1), all-ADD form:
  ta_c = [zr_c*wr_c | zr_c*wi_c]    (DVE, broadcast-read zr_c from PSUM)
  tb_c = [-zi_c*wi_c | zi_c*wr_c]   (DVE, broadcast-read zi_c from PSUM)
  dst_c = ta_c + tb_c = [re_c|im_c] (GpSimd)
"""

import os
from contextlib import ExitStack

import numpy as np

import concourse.bass as bass
import concourse.mybir as mybir
import concourse.tile as tile
from concourse import bacc
from concourse.bass_utils import run_bass_kernel_spmd

B, TLEN = 256, 32768
NFFT, R = 65536, 256
NCORES = 8
RPC = B // NCORES      # rows per core = 32
NPAIR = RPC // 2       # row-pairs per core = 16

F32 = mybir.dt.float32
ADD = mybir.AluOpType.add
MUL = mybir.AluOpType.mult

# matmul dtype: "f32" (exact, 4 cyc/row) or "f32r" (1 cyc/row, ~2.5e-4 rel err)
MM_DTYPE = os.environ.get("EFF_MM_DTYPE", "f32r")

LAST_RESULT = None  # BassKernelResults of the most recent run (for test.py)

_PROGRAM_CACHE = {}


def _wpair_cm(wr, wi):
    """Chunk-major all-ADD grids from (256,256) wr/wi:
    wa = [wr_c0|wi_c0|wr_c1|wi_c1], wb = [-wi_c0|wr_c0|-wi_c1|wr_c1],
    each (128,1024) f32, where chunk c = rows [128c, 128c+128)."""
    wa = np.concatenate([wr[:128], wi[:128], wr[128:], wi[128:]], axis=1)
    wb = np.concatenate([-wi[:128], wr[:128], -wi[128:], wr[128:]], axis=1)
    return (np.ascontiguousarray(wa, dtype=np.float32),
            np.ascontiguousarray(wb, dtype=np.float32))


def _static_consts():
    k = np.arange(R, dtype=np.float64)
    ang_r = 2 * np.pi * np.outer(k, k) / R
    fr = np.cos(ang_r)
    fi = -np.sin(ang_r)            # F = exp(-2*pi*i*j*k/R)
    ang_n = 2 * np.pi * np.outer(k, k) / NFFT
    c = {
        "fr0": fr[:128, :], "fr1": fr[128:, :],
        "fi0": fi[:128, :], "fi1": fi[128:, :],
        "nfi0": -fi[:128, :], "nfi1": -fi[128:, :],
    }
    c = {n: np.ascontiguousarray(v, dtype=np.float32) for n, v in c.items()}
    c["tfa"], c["tfb"] = _wpair_cm(np.cos(ang_n), -np.sin(ang_n))       # [n2,k1]
    c["tia"], c["tib"] = _wpair_cm(np.cos(ang_n) / NFFT, np.sin(ang_n) / NFFT)
    return c


def _filter_spectrum(filt):
    fpad = np.zeros(NFFT, dtype=np.float64)
    fpad[:TLEN] = filt.reshape(-1).astype(np.float64)
    s = np.fft.fft(fpad)
    st = s.reshape(R, R)           # [k2, k1] since S[k1 + 256*k2]
    return _wpair_cm(st.real, st.imag)


def _build_program(mm_dtype_name):
    nc = bacc.Bacc()
    MMDT = mybir.dt.float32r if mm_dtype_name == "f32r" else F32
    x = nc.declare_dram_parameter("x", [RPC, TLEN], MMDT, isOutput=False)
    y = nc.declare_dram_parameter("y", [RPC, TLEN], F32, isOutput=True)
    small = ["fr0", "fr1", "fi0", "fi1", "nfi0", "nfi1"]
    big = ["tfa", "tfb", "spa", "spb", "tia", "tib"]
    dram = {n: nc.declare_dram_parameter(n, [128, 256], MMDT, isOutput=False)
            for n in small}
    dram.update({n: nc.declare_dram_parameter(n, [128, 1024], F32, isOutput=False)
                 for n in big})

    xg = x.rearrange("b (p q) -> b p q", p=128)   # row -> (128,256) grid
    yg = y.rearrange("b (p q) -> b p q", p=128)

    with ExitStack() as ctx:
        tc = ctx.enter_context(tile.TileContext(nc))
        cpool = ctx.enter_context(tc.tile_pool(name="consts", bufs=1))
        C = {}
        # order: earliest-needed consts first so rp0 can start ASAP
        order = [("fr0", 256), ("fi0", 256), ("nfi0", 256),
                 ("tfa", 1024), ("tfb", 1024),
                 ("fr1", 256), ("fi1", 256), ("nfi1", 256),
                 ("spa", 1024), ("spb", 1024), ("tia", 1024), ("tib", 1024)]
        for n, w in order:
            dt_ = MMDT if w == 256 else F32
            C[n] = cpool.tile([128, w], dt_, tag=n, name=n)
            nc.sync.dma_start(C[n][:], dram[n][:])

        apool = ctx.enter_context(tc.tile_pool(name="a", bufs=6))
        tpool = ctx.enter_context(tc.tile_pool(name="tmp", bufs=3))
        sbpool = ctx.enter_context(tc.tile_pool(name="sb", bufs=2))
        ypool = ctx.enter_context(tc.tile_pool(name="yout", bufs=3))
        pspool = ctx.enter_context(tc.tile_pool(name="ps", bufs=1, space="PSUM"))

        def cmul_evac(dst, src_re, src_im, wa, wb, pref):
            """dst (128,1024) chunk-major [re0|im0|re1|im1] =
            (src_re + i src_im) * w, per-chunk all-ADD form.
            src_* in PSUM (128,512); wa/wb (128,1024) chunk-major const."""
            for cch in range(2):
                ta = tpool.tile([128, 512], F32, tag=f"{pref}_ta", name=f"{pref}_ta")
                tb = tpool.tile([128, 512], F32, tag=f"{pref}_tb", name=f"{pref}_tb")
                csl = slice(256 * cch, 256 * cch + 256)
                wsl = slice(512 * cch, 512 * cch + 512)
                sr = src_re[:, csl][:, None, :].to_broadcast([128, 2, 256])
                si = src_im[:, csl][:, None, :].to_broadcast([128, 2, 256])

                def v3(t):
                    return t.rearrange("p (a q) -> p a q", a=2)

                nc.vector.tensor_tensor(v3(ta[:]), sr, v3(wa[:, wsl]), MUL)
                nc.vector.tensor_tensor(v3(tb[:]), si, v3(wb[:, wsl]), MUL)
                nc.gpsimd.tensor_tensor(dst[:, wsl], ta[:], tb[:], ADD)

        # chunk-major slices of a (128,1024) complex intermediate
        def cre(t, cch):
            return t[:, 512 * cch:512 * cch + 256]

        def cim(t, cch):
            return t[:, 512 * cch + 256:512 * cch + 512]

        state = {}

        def emit_s1_tw1(p):
            ar = apool.tile([128, 256], MMDT, tag="ar", name="ar")
            ai = apool.tile([128, 256], MMDT, tag="ai", name="ai")
            nc.sync.dma_start(ar[:], xg[2 * p])
            nc.sync.dma_start(ai[:], xg[2 * p + 1])

            btre = pspool.tile([128, 512], F32, tag="btre", name="btre")
            btim = pspool.tile([128, 512], F32, tag="btim", name="btim")
            for n2c in range(2):
                asl = slice(128 * n2c, 128 * n2c + 128)
                dre = btre[:, 256 * n2c:256 * n2c + 256]
                dim = btim[:, 256 * n2c:256 * n2c + 256]
                nc.tensor.matmul(dre, ar[:, asl], C["fr0"][:],
                                 start=True, stop=False)
                nc.tensor.matmul(dre, ai[:, asl], C["nfi0"][:],
                                 start=False, stop=True)
                nc.tensor.matmul(dim, ar[:, asl], C["fi0"][:],
                                 start=True, stop=False)
                nc.tensor.matmul(dim, ai[:, asl], C["fr0"][:],
                                 start=False, stop=True)

            ct = sbpool.tile([128, 1024], MMDT, tag="ct", name="ct")
            cmul_evac(ct, btre, btim, C["tfa"], C["tfb"], "tw1")
            state[("ct", p)] = ct

        def emit_s2_sp(p):
            ct = state.pop(("ct", p))
            dtre = pspool.tile([128, 512], F32, tag="dtre", name="dtre")
            dtim = pspool.tile([128, 512], F32, tag="dtim", name="dtim")
            for k2c in range(2):
                fsl = slice(128 * k2c, 128 * k2c + 128)
                dre = dtre[:, 256 * k2c:256 * k2c + 256]
                dim = dtim[:, 256 * k2c:256 * k2c + 256]
                for n2c in range(2):
                    frn = C[f"fr{n2c}"][:, fsl]
                    fin = C[f"fi{n2c}"][:, fsl]
                    nfin = C[f"nfi{n2c}"][:, fsl]
                    nc.tensor.matmul(dre, frn, cre(ct, n2c),
                                     start=(n2c == 0), stop=False)
                    nc.tensor.matmul(dre, nfin, cim(ct, n2c),
                                     start=False, stop=(n2c == 1))
                    nc.tensor.matmul(dim, frn, cim(ct, n2c),
                                     start=(n2c == 0), stop=False)
                    nc.tensor.matmul(dim, fin, cre(ct, n2c),
                                     start=False, stop=(n2c == 1))

            pt = sbpool.tile([128, 1024], MMDT, tag="pt", name="pt")
            cmul_evac(pt, dtre, dtim, C["spa"], C["spb"], "sp")
            state[("pt", p)] = pt

        def emit_s3_tw2(p):
            pt = state.pop(("pt", p))
            ere = pspool.tile([128, 512], F32, tag="ere", name="ere")
            eim = pspool.tile([128, 512], F32, tag="eim", name="eim")
            # E_re = sum Pr*Fr + Pi*Fi ; E_im = sum -Pr*Fi + Pi*Fr
            for k1c in range(2):
                dre = ere[:, 256 * k1c:256 * k1c + 256]
                dim = eim[:, 256 * k1c:256 * k1c + 256]
                for k2c in range(2):
                    pres = pt[:, 512 * k2c + 128 * k1c:
                              512 * k2c + 128 * k1c + 128]
                    pims = pt[:, 512 * k2c + 256 + 128 * k1c:
                              512 * k2c + 256 + 128 * k1c + 128]
                    nc.tensor.matmul(dre, pres, C[f"fr{k2c}"][:],
                                     start=(k2c == 0), stop=False)
                    nc.tensor.matmul(dre, pims, C[f"fi{k2c}"][:],
                                     start=False, stop=(k2c == 1))
                    nc.tensor.matmul(dim, pres, C[f"nfi{k2c}"][:],
                                     start=(k2c == 0), stop=False)
                    nc.tensor.matmul(dim, pims, C[f"fr{k2c}"][:],
                                     start=False, stop=(k2c == 1))

            g = sbpool.tile([128, 1024], MMDT, tag="g", name="g")
            cmul_evac(g, ere, eim, C["tia"], C["tib"], "tw2")
            state[("g", p)] = g

        def emit_s4_out(p):
            g = state.pop(("g", p))
            # (yre and yim share one PSUM bank: the two accumulation groups
            # must be sequential, not interleaved)
            yps = pspool.tile([128, 512], F32, tag="yps", name="yps")
            yre = yps[:, 0:256]
            yim = yps[:, 256:512]
            # Y_re = sum Fr.T Gr + Fi.T Gi ; Y_im = sum Fr.T Gi - Fi.T Gr
            for k1c in range(2):
                frh = C[f"fr{k1c}"][:, 0:128]
                fih = C[f"fi{k1c}"][:, 0:128]
                nc.tensor.matmul(yre, frh, cre(g, k1c),
                                 start=(k1c == 0), stop=False)
                nc.tensor.matmul(yre, fih, cim(g, k1c),
                                 start=False, stop=(k1c == 1))
            for k1c in range(2):
                frh = C[f"fr{k1c}"][:, 0:128]
                nfih = C[f"nfi{k1c}"][:, 0:128]
                nc.tensor.matmul(yim, frh, cim(g, k1c),
                                 start=(k1c == 0), stop=False)
                nc.tensor.matmul(yim, nfih, cre(g, k1c),
                                 start=False, stop=(k1c == 1))

            ysb = ypool.tile([128, 512], F32, tag="ysb", name="ysb")
            nc.scalar.copy(ysb[:], yps[:])
            nc.sync.dma_start(yg[2 * p], ysb[:, 0:256])
            nc.sync.dma_start(yg[2 * p + 1], ysb[:, 256:512])

        # software pipeline: skew pair p-1's back half between pair p's halves
        for p in range(NPAIR + 1):
            if p < NPAIR:
                emit_s1_tw1(p)
            if p >= 1:
                emit_s3_tw2(p - 1)
            if p < NPAIR:
                emit_s2_sp(p)
            if p >= 1:
                emit_s4_out(p - 1)

    nc.compile()
    return nc


def _get_program(mm_dtype_name):
    if mm_dtype_name not in _PROGRAM_CACHE:
        _PROGRAM_CACHE[mm_dtype_name] = _build_program(mm_dtype_name)
    return _PROGRAM_CACHE[mm_dtype_name]


def kernel(x, filt):
    global LAST_RESULT
    assert x.shape == (B, TLEN) and x.dtype == np.float32
    nc = _get_program(MM_DTYPE)

    consts = dict(_static_consts())
    consts["spa"], consts["spb"] = _filter_spectrum(filt)

    in_maps = []
    for c in range(NCORES):
        m = dict(consts)
        m["x"] = np.ascontiguousarray(x[c * RPC:(c + 1) * RPC])
        in_maps.append(m)

    res = run_bass_kernel_spmd(
        nc, in_maps, core_ids=list(range(NCORES)),
        trace=bool(int(os.environ.get("EFF_TRACE", "0"))),
    )
    LAST_RESULT = res
    return np.concatenate([r["y"] for r in res.results], axis=0)


# revision 9
# speedup vs baseline: 2.8574x; 1.2272x over previous
"""Trainium2 Bass kernel for EpochedFutureFill: exact causal convolution
y[b,t] = sum_s filt[s] x[b,t-s], computed via a 65536-point FFT per row.

Strategy (data-parallel over 8 cores, 32 rows each):
  - Pack two real rows per complex signal: z = x_a + i*x_b. Since
    IFFT(S * FFT(z)) is linear over R^2, Re -> conv(x_a), Im -> conv(x_b).
  - 65536-pt FFT as 2-stage Cooley-Tukey (256 x 256) built from 256-pt DFT
    matmuls on the tensor engine (float32r mode: full-rate fp32 matmul);
    twiddle/spectrum pointwise multiplies on the vector engine with
    broadcast reads + gpsimd for the SBUF-side combines.
  - Matmul orientations chosen so no transposes are ever needed; the
    filter spectrum (tiny: 1 row) and all DFT/twiddle matrices are
    precomputed host-side and passed as constants.
  - Row-pairs are software-pipelined: emission order skews the second half
    of pair p-1 between the two halves of pair p, so each engine always
    has independent ready work.

Per row-pair dataflow (256x256 grids; complex SBUF intermediates are
(128,1024) chunk-major: [re_c0(256)|im_c0|re_c1|im_c1]):
  A[n1,n2] = z[n1*256+n2]            (n1>=128 is zero padding -> skipped)
  S1: B^T[n2,k1] = sum_n1 A[n1,n2] F[n1,k1]      (A-stationary)
  tw1: C^T = B^T * W_N^{k1 n2}                   (DVE+POOL)
  S2: D^T[k2,k1] = sum_n2 F[n2,k2] C^T[n2,k1]    (F-stationary)
  sp: P^T = D^T * S^T                            (DVE+POOL)
  S3: E[k1,nL] = sum_k2 P^T[k2,k1] conjF[k2,nL]  (P^T-stationary)
  tw2: G = E * W_N^{-nL k1} / N                  (DVE+POOL)
  S4: Y[nH,nL] = sum_k1 conjF[k1,nH] G[k1,nL]    (nH<128 only)
  y_a = Re(Y).flatten(), y_b = Im(Y).flatten()

Each pointwise complex multiply z*w is per-chunk (c = 0,1), all-ADD form:
  ta_c = [zr_c*wr_c | zr_c*wi_c]    (DVE, broadcast-read zr_c from PSUM)
  tb_c = [-zi_c*wi_c | zi_c*wr_c]   (DVE, broadcast-read zi_c from PSUM)
  dst_c = ta_c + tb_c = [re_c|im_c] (GpSimd)
"""

import os
from contextlib import ExitStack

import numpy as np

import concourse.bass as bass
import concourse.mybir as mybir
import concourse.tile as tile
from concourse import bacc
from concourse.bass_utils import run_bass_kernel_spmd

B, TLEN = 256, 32768
NFFT, R = 65536, 256
NCORES = 8
RPC = B // NCORES      # rows per core = 32
NPAIR = RPC // 2       # row-pairs per core = 16

F32 = mybir.dt.float32
ADD = mybir.AluOpType.add
MUL = mybir.AluOpType.mult

# matmul dtype: "f32" (exact, 4 cyc/row) or "f32r" (1 cyc/row, ~2.5e-4 rel err)
MM_DTYPE = os.environ.get("EFF_MM_DTYPE", "f32r")

LAST_RESULT = None  # BassKernelResults of the most recent run (for test.py)

_PROGRAM_CACHE = {}


def _mrg(m):
    """(256,256) -> (128,512): [:, :256] = rows 0:128, [:, 256:] = rows 128:256."""
    return np.concatenate([m[:128, :], m[128:, :]], axis=1)


def _wpair_cm(wr, wi):
    """All-ADD grids: wa = [mrg(wr)|mrg(wi)], wb = [mrg(-wi)|mrg(wr)],
    each (128,1024) f32."""
    wa = np.concatenate([_mrg(wr), _mrg(wi)], axis=1)
    wb = np.concatenate([_mrg(-wi), _mrg(wr)], axis=1)
    return (np.ascontiguousarray(wa, dtype=np.float32),
            np.ascontiguousarray(wb, dtype=np.float32))


def _static_consts():
    k = np.arange(R, dtype=np.float64)
    ang_r = 2 * np.pi * np.outer(k, k) / R
    fr = np.cos(ang_r)
    fi = -np.sin(ang_r)            # F = exp(-2*pi*i*j*k/R)
    ang_n = 2 * np.pi * np.outer(k, k) / NFFT
    c = {
        "fr0": fr[:128, :], "fr1": fr[128:, :],
        "fi0": fi[:128, :], "fi1": fi[128:, :],
        "nfi0": -fi[:128, :], "nfi1": -fi[128:, :],
    }
    c = {n: np.ascontiguousarray(v, dtype=np.float32) for n, v in c.items()}
    c["tfa"], c["tfb"] = _wpair_cm(np.cos(ang_n), -np.sin(ang_n))       # [n2,k1]
    c["tia"], c["tib"] = _wpair_cm(np.cos(ang_n) / NFFT, np.sin(ang_n) / NFFT)
    return c


def _filter_spectrum(filt):
    fpad = np.zeros(NFFT, dtype=np.float64)
    fpad[:TLEN] = filt.reshape(-1).astype(np.float64)
    s = np.fft.fft(fpad)
    st = s.reshape(R, R)           # [k2, k1] since S[k1 + 256*k2]
    return _wpair_cm(st.real, st.imag)


def _build_program(mm_dtype_name):
    nc = bacc.Bacc()
    MMDT = mybir.dt.float32r if mm_dtype_name == "f32r" else F32
    x = nc.declare_dram_parameter("x", [RPC, TLEN], MMDT, isOutput=False)
    y = nc.declare_dram_parameter("y", [RPC, TLEN], F32, isOutput=True)
    small = ["fr0", "fr1", "fi0", "fi1", "nfi0", "nfi1"]
    big = ["tfa", "tfb", "spa", "spb", "tia", "tib"]
    dram = {n: nc.declare_dram_parameter(n, [128, 256], MMDT, isOutput=False)
            for n in small}
    dram.update({n: nc.declare_dram_parameter(n, [128, 1024], F32, isOutput=False)
                 for n in big})

    xg = x.rearrange("b (p q) -> b p q", p=128)   # row -> (128,256) grid
    yg = y.rearrange("b (p q) -> b p q", p=128)

    with ExitStack() as ctx:
        tc = ctx.enter_context(tile.TileContext(nc))
        cpool = ctx.enter_context(tc.tile_pool(name="consts", bufs=1))
        C = {}
        # order: earliest-needed consts first so rp0 can start ASAP
        order = [("fr0", 256), ("fi0", 256), ("nfi0", 256),
                 ("tfa", 1024), ("tfb", 1024),
                 ("fr1", 256), ("fi1", 256), ("nfi1", 256),
                 ("spa", 1024), ("spb", 1024), ("tia", 1024), ("tib", 1024)]
        for n, w in order:
            dt_ = MMDT if w == 256 else F32
            C[n] = cpool.tile([128, w], dt_, tag=n, name=n)
            nc.sync.dma_start(C[n][:], dram[n][:])

        apool = ctx.enter_context(tc.tile_pool(name="a", bufs=6))
        tpool = ctx.enter_context(tc.tile_pool(name="tmp", bufs=4))
        sbpool = ctx.enter_context(tc.tile_pool(name="sb", bufs=3))
        ypool = ctx.enter_context(tc.tile_pool(name="yout", bufs=3))
        pspool = ctx.enter_context(tc.tile_pool(name="ps", bufs=1, space="PSUM"))

        def cmul_evac(dst, src_re, src_im, wa, wb, pref):
            """dst (128,1024) = [re(512)|im(512)] = (src_re + i src_im) * w,
            all-ADD form. src_* in PSUM (128,512); wa/wb (128,1024) const."""
            ta = tpool.tile([128, 1024], F32, tag=f"{pref}_ta", name=f"{pref}_ta")
            tb = tpool.tile([128, 1024], F32, tag=f"{pref}_tb", name=f"{pref}_tb")
            sr = src_re[:, None, :].to_broadcast([128, 2, 512])
            si = src_im[:, None, :].to_broadcast([128, 2, 512])

            def v3(t):
                return t.rearrange("p (a q) -> p a q", a=2)

            nc.vector.tensor_tensor(v3(ta[:]), sr, v3(wa[:]), MUL)
            nc.vector.tensor_tensor(v3(tb[:]), si, v3(wb[:]), MUL)
            nc.gpsimd.tensor_tensor(dst[:], ta[:], tb[:], ADD)

        # slices of a (128,1024) [re(512)|im(512)] complex intermediate
        def cre(t, cch):
            return t[:, 256 * cch:256 * cch + 256]

        def cim(t, cch):
            return t[:, 512 + 256 * cch:512 + 256 * cch + 256]

        state = {}

        def emit_s1_tw1(p):
            ar = apool.tile([128, 256], MMDT, tag="ar", name="ar")
            ai = apool.tile([128, 256], MMDT, tag="ai", name="ai")
            nc.sync.dma_start(ar[:], xg[2 * p])
            nc.sync.dma_start(ai[:], xg[2 * p + 1])

            btre = pspool.tile([128, 512], F32, tag="btre", name="btre")
            btim = pspool.tile([128, 512], F32, tag="btim", name="btim")
            for n2c in range(2):
                asl = slice(128 * n2c, 128 * n2c + 128)
                dre = btre[:, 256 * n2c:256 * n2c + 256]
                dim = btim[:, 256 * n2c:256 * n2c + 256]
                nc.tensor.matmul(dre, ar[:, asl], C["fr0"][:],
                                 start=True, stop=False)
                nc.tensor.matmul(dre, ai[:, asl], C["nfi0"][:],
                                 start=False, stop=True)
                nc.tensor.matmul(dim, ar[:, asl], C["fi0"][:],
                                 start=True, stop=False)
                nc.tensor.matmul(dim, ai[:, asl], C["fr0"][:],
                                 start=False, stop=True)

            ct = sbpool.tile([128, 1024], MMDT, tag="ct", name="ct")
            cmul_evac(ct, btre, btim, C["tfa"], C["tfb"], "tw1")
            state[("ct", p)] = ct

        def emit_s2_sp(p):
            ct = state.pop(("ct", p))
            dtre = pspool.tile([128, 512], F32, tag="dtre", name="dtre")
            dtim = pspool.tile([128, 512], F32, tag="dtim", name="dtim")
            for k2c in range(2):
                fsl = slice(128 * k2c, 128 * k2c + 128)
                dre = dtre[:, 256 * k2c:256 * k2c + 256]
                dim = dtim[:, 256 * k2c:256 * k2c + 256]
                for n2c in range(2):
                    frn = C[f"fr{n2c}"][:, fsl]
                    fin = C[f"fi{n2c}"][:, fsl]
                    nfin = C[f"nfi{n2c}"][:, fsl]
                    nc.tensor.matmul(dre, frn, cre(ct, n2c),
                                     start=(n2c == 0), stop=False)
                    nc.tensor.matmul(dre, nfin, cim(ct, n2c),
                                     start=False, stop=(n2c == 1))
                    nc.tensor.matmul(dim, frn, cim(ct, n2c),
                                     start=(n2c == 0), stop=False)
                    nc.tensor.matmul(dim, fin, cre(ct, n2c),
                                     start=False, stop=(n2c == 1))

            pt = sbpool.tile([128, 1024], MMDT, tag="pt", name="pt")
            cmul_evac(pt, dtre, dtim, C["spa"], C["spb"], "sp")
            state[("pt", p)] = pt

        def emit_s3_tw2(p):
            pt = state.pop(("pt", p))
            ere = pspool.tile([128, 512], F32, tag="ere", name="ere")
            eim = pspool.tile([128, 512], F32, tag="eim", name="eim")
            # E_re = sum Pr*Fr + Pi*Fi ; E_im = sum -Pr*Fi + Pi*Fr
            for k1c in range(2):
                dre = ere[:, 256 * k1c:256 * k1c + 256]
                dim = eim[:, 256 * k1c:256 * k1c + 256]
                for k2c in range(2):
                    pres = pt[:, 256 * k2c + 128 * k1c:
                              256 * k2c + 128 * k1c + 128]
                    pims = pt[:, 512 + 256 * k2c + 128 * k1c:
                              512 + 256 * k2c + 128 * k1c + 128]
                    nc.tensor.matmul(dre, pres, C[f"fr{k2c}"][:],
                                     start=(k2c == 0), stop=False)
                    nc.tensor.matmul(dre, pims, C[f"fi{k2c}"][:],
                                     start=False, stop=(k2c == 1))
                    nc.tensor.matmul(dim, pres, C[f"nfi{k2c}"][:],
                                     start=(k2c == 0), stop=False)
                    nc.tensor.matmul(dim, pims, C[f"fr{k2c}"][:],
                                     start=False, stop=(k2c == 1))

            g = sbpool.tile([128, 1024], MMDT, tag="g", name="g")
            cmul_evac(g, ere, eim, C["tia"], C["tib"], "tw2")
            state[("g", p)] = g

        def emit_s4_out(p):
            g = state.pop(("g", p))
            # (yre and yim share one PSUM bank: the two accumulation groups
            # must be sequential, not interleaved)
            yps = pspool.tile([128, 512], F32, tag="yps", name="yps")
            yre = yps[:, 0:256]
            yim = yps[:, 256:512]
            # Y_re = sum Fr.T Gr + Fi.T Gi ; Y_im = sum Fr.T Gi - Fi.T Gr
            for k1c in range(2):
                frh = C[f"fr{k1c}"][:, 0:128]
                fih = C[f"fi{k1c}"][:, 0:128]
                nc.tensor.matmul(yre, frh, cre(g, k1c),
                                 start=(k1c == 0), stop=False)
                nc.tensor.matmul(yre, fih, cim(g, k1c),
                                 start=False, stop=(k1c == 1))
            for k1c in range(2):
                frh = C[f"fr{k1c}"][:, 0:128]
                nfih = C[f"nfi{k1c}"][:, 0:128]
                nc.tensor.matmul(yim, frh, cim(g, k1c),
                                 start=(k1c == 0), stop=False)
                nc.tensor.matmul(yim, nfih, cre(g, k1c),
                                 start=False, stop=(k1c == 1))

            ysb = ypool.tile([128, 512], F32, tag="ysb", name="ysb")
            nc.scalar.copy(ysb[:], yps[:])
            nc.sync.dma_start(yg[2 * p], ysb[:, 0:256])
            nc.sync.dma_start(yg[2 * p + 1], ysb[:, 256:512])

        # 4-deep software pipeline: every stage consumes data produced a
        # full step earlier, so no engine ever waits on an in-step evac.
        for p in range(NPAIR + 3):
            if p < NPAIR:
                emit_s1_tw1(p)
            if 1 <= p < NPAIR + 1:
                emit_s2_sp(p - 1)
            if 2 <= p < NPAIR + 2:
                emit_s3_tw2(p - 2)
            if p >= 3:
                emit_s4_out(p - 3)

    nc.compile()
    return nc


def _get_program(mm_dtype_name):
    if mm_dtype_name not in _PROGRAM_CACHE:
        _PROGRAM_CACHE[mm_dtype_name] = _build_program(mm_dtype_name)
    return _PROGRAM_CACHE[mm_dtype_name]


def kernel(x, filt):
    global LAST_RESULT
    assert x.shape == (B, TLEN) and x.dtype == np.float32
    nc = _get_program(MM_DTYPE)

    consts = dict(_static_consts())
    consts["spa"], consts["spb"] = _filter_spectrum(filt)

    in_maps = []
    for c in range(NCORES):
        m = dict(consts)
        m["x"] = np.ascontiguousarray(x[c * RPC:(c + 1) * RPC])
        in_maps.append(m)

    res = run_bass_kernel_spmd(
        nc, in_maps, core_ids=list(range(NCORES)),
        trace=bool(int(os.environ.get("EFF_TRACE", "0"))),
    )
    LAST_RESULT = res
    return np.concatenate([r["y"] for r in res.results], axis=0)


# revision 10
# speedup vs baseline: 2.9868x; 1.0453x over previous
"""Trainium2 Bass kernel for EpochedFutureFill: exact causal convolution
y[b,t] = sum_s filt[s] x[b,t-s], computed via a 65536-point FFT per row.

Strategy (data-parallel over 8 cores, 32 rows each):
  - Pack two real rows per complex signal: z = x_a + i*x_b. Since
    IFFT(S * FFT(z)) is linear over R^2, Re -> conv(x_a), Im -> conv(x_b).
  - 65536-pt FFT as 2-stage Cooley-Tukey (256 x 256) built from 256-pt DFT
    matmuls on the tensor engine (float32r mode: full-rate fp32 matmul);
    twiddle/spectrum pointwise multiplies on the vector engine with
    broadcast reads + gpsimd for the SBUF-side combines.
  - Matmul orientations chosen so no transposes are ever needed; the
    filter spectrum (tiny: 1 row) and all DFT/twiddle matrices are
    precomputed host-side and passed as constants.
  - Row-pairs are software-pipelined: emission order skews the second half
    of pair p-1 between the two halves of pair p, so each engine always
    has independent ready work.

Per row-pair dataflow (256x256 grids; complex SBUF intermediates are
(128,1024) chunk-major: [re_c0(256)|im_c0|re_c1|im_c1]):
  A[n1,n2] = z[n1*256+n2]            (n1>=128 is zero padding -> skipped)
  S1: B^T[n2,k1] = sum_n1 A[n1,n2] F[n1,k1]      (A-stationary)
  tw1: C^T = B^T * W_N^{k1 n2}                   (DVE+POOL)
  S2: D^T[k2,k1] = sum_n2 F[n2,k2] C^T[n2,k1]    (F-stationary)
  sp: P^T = D^T * S^T                            (DVE+POOL)
  S3: E[k1,nL] = sum_k2 P^T[k2,k1] conjF[k2,nL]  (P^T-stationary)
  tw2: G = E * W_N^{-nL k1} / N                  (DVE+POOL)
  S4: Y[nH,nL] = sum_k1 conjF[k1,nH] G[k1,nL]    (nH<128 only)
  y_a = Re(Y).flatten(), y_b = Im(Y).flatten()

Each pointwise complex multiply z*w is per-chunk (c = 0,1), all-ADD form:
  ta_c = [zr_c*wr_c | zr_c*wi_c]    (DVE, broadcast-read zr_c from PSUM)
  tb_c = [-zi_c*wi_c | zi_c*wr_c]   (DVE, broadcast-read zi_c from PSUM)
  dst_c = ta_c + tb_c = [re_c|im_c] (GpSimd)
"""

import os
from contextlib import ExitStack

import numpy as np

import concourse.bass as bass
import concourse.mybir as mybir
import concourse.tile as tile
from concourse import bacc
from concourse.bass_utils import run_bass_kernel_spmd

B, TLEN = 256, 32768
NFFT, R = 65536, 256
NCORES = 8
RPC = B // NCORES      # rows per core = 32
NPAIR = RPC // 2       # row-pairs per core = 16

F32 = mybir.dt.float32
ADD = mybir.AluOpType.add
MUL = mybir.AluOpType.mult

# matmul dtype: "f32" (exact, 4 cyc/row) or "f32r" (1 cyc/row, ~2.5e-4 rel err)
MM_DTYPE = os.environ.get("EFF_MM_DTYPE", "f32r")

LAST_RESULT = None  # BassKernelResults of the most recent run (for test.py)

_PROGRAM_CACHE = {}


def _mrg(m):
    """(256,256) -> (128,512): [:, :256] = rows 0:128, [:, 256:] = rows 128:256."""
    return np.concatenate([m[:128, :], m[128:, :]], axis=1)


def _wpair_cm(wr, wi):
    """All-ADD grids: wa = [mrg(wr)|mrg(wi)], wb = [mrg(-wi)|mrg(wr)],
    each (128,1024) f32."""
    wa = np.concatenate([_mrg(wr), _mrg(wi)], axis=1)
    wb = np.concatenate([_mrg(-wi), _mrg(wr)], axis=1)
    return (np.ascontiguousarray(wa, dtype=np.float32),
            np.ascontiguousarray(wb, dtype=np.float32))


def _static_consts():
    k = np.arange(R, dtype=np.float64)
    ang_r = 2 * np.pi * np.outer(k, k) / R
    fr = np.cos(ang_r)
    fi = -np.sin(ang_r)            # F = exp(-2*pi*i*j*k/R)
    ang_n = 2 * np.pi * np.outer(k, k) / NFFT
    c = {
        "fr0": fr[:128, :], "fr1": fr[128:, :],
        "fi0": fi[:128, :], "fi1": fi[128:, :],
        "nfi0": -fi[:128, :], "nfi1": -fi[128:, :],
    }
    c = {n: np.ascontiguousarray(v, dtype=np.float32) for n, v in c.items()}
    c["tfa"], c["tfb"] = _wpair_cm(np.cos(ang_n), -np.sin(ang_n))       # [n2,k1]
    c["tia"], c["tib"] = _wpair_cm(np.cos(ang_n) / NFFT, np.sin(ang_n) / NFFT)
    return c


def _filter_spectrum(filt):
    fpad = np.zeros(NFFT, dtype=np.float64)
    fpad[:TLEN] = filt.reshape(-1).astype(np.float64)
    s = np.fft.fft(fpad)
    st = s.reshape(R, R)           # [k2, k1] since S[k1 + 256*k2]
    return _wpair_cm(st.real, st.imag)


def _build_program(mm_dtype_name):
    nc = bacc.Bacc()
    MMDT = mybir.dt.float32r if mm_dtype_name == "f32r" else F32
    x = nc.declare_dram_parameter("x", [RPC, TLEN], MMDT, isOutput=False)
    y = nc.declare_dram_parameter("y", [RPC, TLEN], F32, isOutput=True)
    small = ["fr0", "fr1", "fi0", "fi1", "nfi0", "nfi1"]
    big = ["tfa", "tfb", "spa", "spb", "tia", "tib"]
    dram = {n: nc.declare_dram_parameter(n, [128, 256], MMDT, isOutput=False)
            for n in small}
    dram.update({n: nc.declare_dram_parameter(n, [128, 1024], F32, isOutput=False)
                 for n in big})

    xg = x.rearrange("b (p q) -> b p q", p=128)   # row -> (128,256) grid
    yg = y.rearrange("b (p q) -> b p q", p=128)

    with ExitStack() as ctx:
        tc = ctx.enter_context(tile.TileContext(nc))
        cpool = ctx.enter_context(tc.tile_pool(name="consts", bufs=1))
        C = {}

        def load_consts(names):
            for n in names:
                w = 1024 if n in big else 256
                dt_ = MMDT if w == 256 else F32
                C[n] = cpool.tile([128, w], dt_, tag=n, name=n)
                nc.sync.dma_start(C[n][:], dram[n][:])

        apool = ctx.enter_context(tc.tile_pool(name="a", bufs=8))

        # DMA issue order = need order: step-0 consts, first input rows,
        # then the rest (spa/spb needed step 1, tia/tib step 2).
        load_consts(["fr0", "fi0", "nfi0"])
        prefetched = {}
        for p in range(3):
            ar = apool.tile([128, 256], MMDT, tag="ar", name="ar")
            ai = apool.tile([128, 256], MMDT, tag="ai", name="ai")
            nc.sync.dma_start(ar[:], xg[2 * p])
            nc.sync.dma_start(ai[:], xg[2 * p + 1])
            prefetched[p] = (ar, ai)
        load_consts(["tfa", "tfb", "fr1", "fi1", "nfi1",
                     "spa", "spb", "tia", "tib"])
        tpool = ctx.enter_context(tc.tile_pool(name="tmp", bufs=4))
        sbpool = ctx.enter_context(tc.tile_pool(name="sb", bufs=3))
        ypool = ctx.enter_context(tc.tile_pool(name="yout", bufs=3))
        pspool = ctx.enter_context(tc.tile_pool(name="ps", bufs=1, space="PSUM"))

        def cmul_evac(dst, src_re, src_im, wa, wb, pref):
            """dst (128,1024) = [re(512)|im(512)] = (src_re + i src_im) * w,
            all-ADD form. src_* in PSUM (128,512); wa/wb (128,1024) const."""
            ta = tpool.tile([128, 1024], F32, tag=f"{pref}_ta", name=f"{pref}_ta")
            tb = tpool.tile([128, 1024], F32, tag=f"{pref}_tb", name=f"{pref}_tb")
            sr = src_re[:, None, :].to_broadcast([128, 2, 512])
            si = src_im[:, None, :].to_broadcast([128, 2, 512])

            def v3(t):
                return t.rearrange("p (a q) -> p a q", a=2)

            nc.vector.tensor_tensor(v3(ta[:]), sr, v3(wa[:]), MUL)
            nc.vector.tensor_tensor(v3(tb[:]), si, v3(wb[:]), MUL)
            nc.gpsimd.tensor_tensor(dst[:], ta[:], tb[:], ADD)

        # slices of a (128,1024) [re(512)|im(512)] complex intermediate
        def cre(t, cch):
            return t[:, 256 * cch:256 * cch + 256]

        def cim(t, cch):
            return t[:, 512 + 256 * cch:512 + 256 * cch + 256]

        state = {}

        def emit_s1_tw1(p):
            if p in prefetched:
                ar, ai = prefetched.pop(p)
            else:
                ar = apool.tile([128, 256], MMDT, tag="ar", name="ar")
                ai = apool.tile([128, 256], MMDT, tag="ai", name="ai")
                nc.sync.dma_start(ar[:], xg[2 * p])
                nc.sync.dma_start(ai[:], xg[2 * p + 1])

            btre = pspool.tile([128, 512], F32, tag="btre", name="btre")
            btim = pspool.tile([128, 512], F32, tag="btim", name="btim")
            for n2c in range(2):
                asl = slice(128 * n2c, 128 * n2c + 128)
                dre = btre[:, 256 * n2c:256 * n2c + 256]
                dim = btim[:, 256 * n2c:256 * n2c + 256]
                nc.tensor.matmul(dre, ar[:, asl], C["fr0"][:],
                                 start=True, stop=False)
                nc.tensor.matmul(dre, ai[:, asl], C["nfi0"][:],
                                 start=False, stop=True)
                nc.tensor.matmul(dim, ar[:, asl], C["fi0"][:],
                                 start=True, stop=False)
                nc.tensor.matmul(dim, ai[:, asl], C["fr0"][:],
                                 start=False, stop=True)

            ct = sbpool.tile([128, 1024], MMDT, tag="ct", name="ct")
            cmul_evac(ct, btre, btim, C["tfa"], C["tfb"], "tw1")
            state[("ct", p)] = ct

        def emit_s2_sp(p):
            ct = state.pop(("ct", p))
            dtre = pspool.tile([128, 512], F32, tag="dtre", name="dtre")
            dtim = pspool.tile([128, 512], F32, tag="dtim", name="dtim")
            for k2c in range(2):
                fsl = slice(128 * k2c, 128 * k2c + 128)
                dre = dtre[:, 256 * k2c:256 * k2c + 256]
                dim = dtim[:, 256 * k2c:256 * k2c + 256]
                for n2c in range(2):
                    frn = C[f"fr{n2c}"][:, fsl]
                    fin = C[f"fi{n2c}"][:, fsl]
                    nfin = C[f"nfi{n2c}"][:, fsl]
                    nc.tensor.matmul(dre, frn, cre(ct, n2c),
                                     start=(n2c == 0), stop=False)
                    nc.tensor.matmul(dre, nfin, cim(ct, n2c),
                                     start=False, stop=(n2c == 1))
                    nc.tensor.matmul(dim, frn, cim(ct, n2c),
                                     start=(n2c == 0), stop=False)
                    nc.tensor.matmul(dim, fin, cre(ct, n2c),
                                     start=False, stop=(n2c == 1))

            pt = sbpool.tile([128, 1024], MMDT, tag="pt", name="pt")
            cmul_evac(pt, dtre, dtim, C["spa"], C["spb"], "sp")
            state[("pt", p)] = pt

        def emit_s3_tw2(p):
            pt = state.pop(("pt", p))
            ere = pspool.tile([128, 512], F32, tag="ere", name="ere")
            eim = pspool.tile([128, 512], F32, tag="eim", name="eim")
            # E_re = sum Pr*Fr + Pi*Fi ; E_im = sum -Pr*Fi + Pi*Fr
            for k1c in range(2):
                dre = ere[:, 256 * k1c:256 * k1c + 256]
                dim = eim[:, 256 * k1c:256 * k1c + 256]
                for k2c in range(2):
                    pres = pt[:, 256 * k2c + 128 * k1c:
                              256 * k2c + 128 * k1c + 128]
                    pims = pt[:, 512 + 256 * k2c + 128 * k1c:
                              512 + 256 * k2c + 128 * k1c + 128]
                    nc.tensor.matmul(dre, pres, C[f"fr{k2c}"][:],
                                     start=(k2c == 0), stop=False)
                    nc.tensor.matmul(dre, pims, C[f"fi{k2c}"][:],
                                     start=False, stop=(k2c == 1))
                    nc.tensor.matmul(dim, pres, C[f"nfi{k2c}"][:],
                                     start=(k2c == 0), stop=False)
                    nc.tensor.matmul(dim, pims, C[f"fr{k2c}"][:],
                                     start=False, stop=(k2c == 1))

            g = sbpool.tile([128, 1024], MMDT, tag="g", name="g")
            cmul_evac(g, ere, eim, C["tia"], C["tib"], "tw2")
            state[("g", p)] = g

        def emit_s4_out(p):
            g = state.pop(("g", p))
            # (yre and yim share one PSUM bank: the two accumulation groups
            # must be sequential, not interleaved)
            yps = pspool.tile([128, 512], F32, tag="yps", name="yps")
            yre = yps[:, 0:256]
            yim = yps[:, 256:512]
            # Y_re = sum Fr.T Gr + Fi.T Gi ; Y_im = sum Fr.T Gi - Fi.T Gr
            for k1c in range(2):
                frh = C[f"fr{k1c}"][:, 0:128]
                fih = C[f"fi{k1c}"][:, 0:128]
                nc.tensor.matmul(yre, frh, cre(g, k1c),
                                 start=(k1c == 0), stop=False)
                nc.tensor.matmul(yre, fih, cim(g, k1c),
                                 start=False, stop=(k1c == 1))
            for k1c in range(2):
                frh = C[f"fr{k1c}"][:, 0:128]
                nfih = C[f"nfi{k1c}"][:, 0:128]
                nc.tensor.matmul(yim, frh, cim(g, k1c),
                                 start=(k1c == 0), stop=False)
                nc.tensor.matmul(yim, nfih, cre(g, k1c),
                                 start=False, stop=(k1c == 1))

            ysb = ypool.tile([128, 512], F32, tag="ysb", name="ysb")
            nc.scalar.copy(ysb[:], yps[:])
            nc.sync.dma_start(yg[2 * p], ysb[:, 0:256])
            nc.sync.dma_start(yg[2 * p + 1], ysb[:, 256:512])

        # 4-deep software pipeline: every stage consumes data produced a
        # full step earlier, so no engine ever waits on an in-step evac.
        for p in range(NPAIR + 3):
            if p < NPAIR:
                emit_s1_tw1(p)
            if 1 <= p < NPAIR + 1:
                emit_s2_sp(p - 1)
            if 2 <= p < NPAIR + 2:
                emit_s3_tw2(p - 2)
            if p >= 3:
                emit_s4_out(p - 3)

    nc.compile()
    return nc


def _get_program(mm_dtype_name):
    if mm_dtype_name not in _PROGRAM_CACHE:
        _PROGRAM_CACHE[mm_dtype_name] = _build_program(mm_dtype_name)
    return _PROGRAM_CACHE[mm_dtype_name]


def kernel(x, filt):
    global LAST_RESULT
    assert x.shape == (B, TLEN) and x.dtype == np.float32
    nc = _get_program(MM_DTYPE)

    consts = dict(_static_consts())
    consts["spa"], consts["spb"] = _filter_spectrum(filt)

    in_maps = []
    for c in range(NCORES):
        m = dict(consts)
        m["x"] = np.ascontiguousarray(x[c * RPC:(c + 1) * RPC])
        in_maps.append(m)

    res = run_bass_kernel_spmd(
        nc, in_maps, core_ids=list(range(NCORES)),
        trace=bool(int(os.environ.get("EFF_TRACE", "0"))),
    )
    LAST_RESULT = res
    return np.concatenate([r["y"] for r in res.results], axis=0)


# revision 11
# speedup vs baseline: 2.9917x; 1.0016x over previous
"""Trainium2 Bass kernel for EpochedFutureFill: exact causal convolution
y[b,t] = sum_s filt[s] x[b,t-s], computed via a 65536-point FFT per row.

Strategy (data-parallel over 8 cores, 32 rows each):
  - Pack two real rows per complex signal: z = x_a + i*x_b. Since
    IFFT(S * FFT(z)) is linear over R^2, Re -> conv(x_a), Im -> conv(x_b).
  - 65536-pt FFT as 2-stage Cooley-Tukey (256 x 256) built from 256-pt DFT
    matmuls on the tensor engine (float32r mode: full-rate fp32 matmul);
    twiddle/spectrum pointwise multiplies on the vector engine with
    broadcast reads + gpsimd for the SBUF-side combines.
  - Matmul orientations chosen so no transposes are ever needed; the
    filter spectrum (tiny: 1 row) and all DFT/twiddle matrices are
    precomputed host-side and passed as constants.
  - Row-pairs are software-pipelined: emission order skews the second half
    of pair p-1 between the two halves of pair p, so each engine always
    has independent ready work.

Per row-pair dataflow (256x256 grids; complex SBUF intermediates are
(128,1024) chunk-major: [re_c0(256)|im_c0|re_c1|im_c1]):
  A[n1,n2] = z[n1*256+n2]            (n1>=128 is zero padding -> skipped)
  S1: B^T[n2,k1] = sum_n1 A[n1,n2] F[n1,k1]      (A-stationary)
  tw1: C^T = B^T * W_N^{k1 n2}                   (DVE+POOL)
  S2: D^T[k2,k1] = sum_n2 F[n2,k2] C^T[n2,k1]    (F-stationary)
  sp: P^T = D^T * S^T                            (DVE+POOL)
  S3: E[k1,nL] = sum_k2 P^T[k2,k1] conjF[k2,nL]  (P^T-stationary)
  tw2: G = E * W_N^{-nL k1} / N                  (DVE+POOL)
  S4: Y[nH,nL] = sum_k1 conjF[k1,nH] G[k1,nL]    (nH<128 only)
  y_a = Re(Y).flatten(), y_b = Im(Y).flatten()

Each pointwise complex multiply z*w is per-chunk (c = 0,1), all-ADD form:
  ta_c = [zr_c*wr_c | zr_c*wi_c]    (DVE, broadcast-read zr_c from PSUM)
  tb_c = [-zi_c*wi_c | zi_c*wr_c]   (DVE, broadcast-read zi_c from PSUM)
  dst_c = ta_c + tb_c = [re_c|im_c] (GpSimd)
"""

import os
from contextlib import ExitStack

import numpy as np

import concourse.bass as bass
import concourse.mybir as mybir
import concourse.tile as tile
from concourse import bacc
from concourse.bass_utils import run_bass_kernel_spmd

B, TLEN = 256, 32768
NFFT, R = 65536, 256
NCORES = 8
RPC = B // NCORES      # rows per core = 32
NPAIR = RPC // 2       # row-pairs per core = 16

F32 = mybir.dt.float32
ADD = mybir.AluOpType.add
MUL = mybir.AluOpType.mult

# matmul dtype: "f32" (exact, 4 cyc/row) or "f32r" (1 cyc/row, ~2.5e-4 rel err)
MM_DTYPE = os.environ.get("EFF_MM_DTYPE", "f32r")

LAST_RESULT = None  # BassKernelResults of the most recent run (for test.py)

_PROGRAM_CACHE = {}


def _mrg(m):
    """(256,256) -> (128,512): [:, :256] = rows 0:128, [:, 256:] = rows 128:256."""
    return np.concatenate([m[:128, :], m[128:, :]], axis=1)


def _wpair_cm(wr, wi):
    """All-ADD grids: wa = [mrg(wr)|mrg(wi)], wb = [mrg(-wi)|mrg(wr)],
    each (128,1024) f32."""
    wa = np.concatenate([_mrg(wr), _mrg(wi)], axis=1)
    wb = np.concatenate([_mrg(-wi), _mrg(wr)], axis=1)
    return (np.ascontiguousarray(wa, dtype=np.float32),
            np.ascontiguousarray(wb, dtype=np.float32))


def _static_consts():
    k = np.arange(R, dtype=np.float64)
    ang_r = 2 * np.pi * np.outer(k, k) / R
    fr = np.cos(ang_r)
    fi = -np.sin(ang_r)            # F = exp(-2*pi*i*j*k/R)
    ang_n = 2 * np.pi * np.outer(k, k) / NFFT
    c = {
        "fr0": fr[:128, :], "fr1": fr[128:, :],
        "fi0": fi[:128, :], "fi1": fi[128:, :],
        "nfi0": -fi[:128, :], "nfi1": -fi[128:, :],
    }
    c = {n: np.ascontiguousarray(v, dtype=np.float32) for n, v in c.items()}
    c["tfa"], c["tfb"] = _wpair_cm(np.cos(ang_n), -np.sin(ang_n))       # [n2,k1]
    c["tia"], c["tib"] = _wpair_cm(np.cos(ang_n) / NFFT, np.sin(ang_n) / NFFT)
    return c


def _filter_spectrum(filt):
    fpad = np.zeros(NFFT, dtype=np.float64)
    fpad[:TLEN] = filt.reshape(-1).astype(np.float64)
    s = np.fft.fft(fpad)
    st = s.reshape(R, R)           # [k2, k1] since S[k1 + 256*k2]
    return _wpair_cm(st.real, st.imag)


def _build_program(mm_dtype_name):
    nc = bacc.Bacc()
    MMDT = mybir.dt.float32r if mm_dtype_name == "f32r" else F32
    x = nc.declare_dram_parameter("x", [RPC, TLEN], MMDT, isOutput=False)
    y = nc.declare_dram_parameter("y", [RPC, TLEN], F32, isOutput=True)
    small = ["fr0", "fr1", "fi0", "fi1", "nfi0", "nfi1"]
    big = ["tfa", "tfb", "spa", "spb", "tia", "tib"]
    dram = {n: nc.declare_dram_parameter(n, [128, 256], MMDT, isOutput=False)
            for n in small}
    dram.update({n: nc.declare_dram_parameter(n, [128, 1024], F32, isOutput=False)
                 for n in big})

    xg = x.rearrange("b (p q) -> b p q", p=128)   # row -> (128,256) grid
    yg = y.rearrange("b (p q) -> b p q", p=128)

    with ExitStack() as ctx:
        tc = ctx.enter_context(tile.TileContext(nc))
        cpool = ctx.enter_context(tc.tile_pool(name="consts", bufs=1))
        C = {}

        def load_consts(names):
            for n in names:
                w = 1024 if n in big else 256
                dt_ = MMDT if w == 256 else F32
                C[n] = cpool.tile([128, w], dt_, tag=n, name=n)
                nc.sync.dma_start(C[n][:], dram[n][:])

        apool = ctx.enter_context(tc.tile_pool(name="a", bufs=8))
        pspool = ctx.enter_context(tc.tile_pool(name="ps", bufs=1, space="PSUM"))

        # DMA issue order = need order: step-0 consts, first input rows,
        # then the rest (spa/spb needed step 1, tia/tib step 2).
        load_consts(["fr0", "fi0", "nfi0"])
        warm = pspool.tile([128, 512], F32, tag="warm", name="warm")
        for _ in range(20):
            nc.tensor.matmul(warm[:, 0:256], C["fr0"][:, 0:128], C["fr0"][:],
                             start=True, stop=True)
        prefetched = {}
        for p in range(3):
            ar = apool.tile([128, 256], MMDT, tag="ar", name="ar")
            ai = apool.tile([128, 256], MMDT, tag="ai", name="ai")
            nc.sync.dma_start(ar[:], xg[2 * p])
            nc.sync.dma_start(ai[:], xg[2 * p + 1])
            prefetched[p] = (ar, ai)
        load_consts(["tfa", "tfb", "fr1", "fi1", "nfi1",
                     "spa", "spb", "tia", "tib"])
        tpool = ctx.enter_context(tc.tile_pool(name="tmp", bufs=4))
        sbpool = ctx.enter_context(tc.tile_pool(name="sb", bufs=3))
        ypool = ctx.enter_context(tc.tile_pool(name="yout", bufs=3))

        def cmul_evac(dst, src_re, src_im, wa, wb, pref):
            """dst (128,1024) = [re(512)|im(512)] = (src_re + i src_im) * w,
            all-ADD form. src_* in PSUM (128,512); wa/wb (128,1024) const."""
            ta = tpool.tile([128, 1024], F32, tag=f"{pref}_ta", name=f"{pref}_ta")
            tb = tpool.tile([128, 1024], F32, tag=f"{pref}_tb", name=f"{pref}_tb")
            sr = src_re[:, None, :].to_broadcast([128, 2, 512])
            si = src_im[:, None, :].to_broadcast([128, 2, 512])

            def v3(t):
                return t.rearrange("p (a q) -> p a q", a=2)

            nc.vector.tensor_tensor(v3(ta[:]), sr, v3(wa[:]), MUL)
            nc.vector.tensor_tensor(v3(tb[:]), si, v3(wb[:]), MUL)
            nc.gpsimd.tensor_tensor(dst[:], ta[:], tb[:], ADD)

        # slices of a (128,1024) [re(512)|im(512)] complex intermediate
        def cre(t, cch):
            return t[:, 256 * cch:256 * cch + 256]

        def cim(t, cch):
            return t[:, 512 + 256 * cch:512 + 256 * cch + 256]

        state = {}

        def emit_s1_tw1(p):
            if p in prefetched:
                ar, ai = prefetched.pop(p)
            else:
                ar = apool.tile([128, 256], MMDT, tag="ar", name="ar")
                ai = apool.tile([128, 256], MMDT, tag="ai", name="ai")
                nc.sync.dma_start(ar[:], xg[2 * p])
                nc.sync.dma_start(ai[:], xg[2 * p + 1])

            btre = pspool.tile([128, 512], F32, tag="btre", name="btre")
            btim = pspool.tile([128, 512], F32, tag="btim", name="btim")
            for n2c in range(2):
                asl = slice(128 * n2c, 128 * n2c + 128)
                dre = btre[:, 256 * n2c:256 * n2c + 256]
                dim = btim[:, 256 * n2c:256 * n2c + 256]
                nc.tensor.matmul(dre, ar[:, asl], C["fr0"][:],
                                 start=True, stop=False)
                nc.tensor.matmul(dre, ai[:, asl], C["nfi0"][:],
                                 start=False, stop=True)
                nc.tensor.matmul(dim, ar[:, asl], C["fi0"][:],
                                 start=True, stop=False)
                nc.tensor.matmul(dim, ai[:, asl], C["fr0"][:],
                                 start=False, stop=True)

            ct = sbpool.tile([128, 1024], MMDT, tag="ct", name="ct")
            cmul_evac(ct, btre, btim, C["tfa"], C["tfb"], "tw1")
            state[("ct", p)] = ct

        def emit_s2_sp(p):
            ct = state.pop(("ct", p))
            dtre = pspool.tile([128, 512], F32, tag="dtre", name="dtre")
            dtim = pspool.tile([128, 512], F32, tag="dtim", name="dtim")
            for k2c in range(2):
                fsl = slice(128 * k2c, 128 * k2c + 128)
                dre = dtre[:, 256 * k2c:256 * k2c + 256]
                dim = dtim[:, 256 * k2c:256 * k2c + 256]
                for n2c in range(2):
                    frn = C[f"fr{n2c}"][:, fsl]
                    fin = C[f"fi{n2c}"][:, fsl]
                    nfin = C[f"nfi{n2c}"][:, fsl]
                    nc.tensor.matmul(dre, frn, cre(ct, n2c),
                                     start=(n2c == 0), stop=False)
                    nc.tensor.matmul(dre, nfin, cim(ct, n2c),
                                     start=False, stop=(n2c == 1))
                    nc.tensor.matmul(dim, frn, cim(ct, n2c),
                                     start=(n2c == 0), stop=False)
                    nc.tensor.matmul(dim, fin, cre(ct, n2c),
                                     start=False, stop=(n2c == 1))

            pt = sbpool.tile([128, 1024], MMDT, tag="pt", name="pt")
            cmul_evac(pt, dtre, dtim, C["spa"], C["spb"], "sp")
            state[("pt", p)] = pt

        def emit_s3_tw2(p):
            pt = state.pop(("pt", p))
            ere = pspool.tile([128, 512], F32, tag="ere", name="ere")
            eim = pspool.tile([128, 512], F32, tag="eim", name="eim")
            # E_re = sum Pr*Fr + Pi*Fi ; E_im = sum -Pr*Fi + Pi*Fr
            for k1c in range(2):
                dre = ere[:, 256 * k1c:256 * k1c + 256]
                dim = eim[:, 256 * k1c:256 * k1c + 256]
                for k2c in range(2):
                    pres = pt[:, 256 * k2c + 128 * k1c:
                              256 * k2c + 128 * k1c + 128]
                    pims = pt[:, 512 + 256 * k2c + 128 * k1c:
                              512 + 256 * k2c + 128 * k1c + 128]
                    nc.tensor.matmul(dre, pres, C[f"fr{k2c}"][:],
                                     start=(k2c == 0), stop=False)
                    nc.tensor.matmul(dre, pims, C[f"fi{k2c}"][:],
                                     start=False, stop=(k2c == 1))
                    nc.tensor.matmul(dim, pres, C[f"nfi{k2c}"][:],
                                     start=(k2c == 0), stop=False)
                    nc.tensor.matmul(dim, pims, C[f"fr{k2c}"][:],
                                     start=False, stop=(k2c == 1))

            g = sbpool.tile([128, 1024], MMDT, tag="g", name="g")
            cmul_evac(g, ere, eim, C["tia"], C["tib"], "tw2")
            state[("g", p)] = g

        def emit_s4_out(p):
            g = state.pop(("g", p))
            # (yre and yim share one PSUM bank: the two accumulation groups
            # must be sequential, not interleaved)
            yps = pspool.tile([128, 512], F32, tag="yps", name="yps")
            yre = yps[:, 0:256]
            yim = yps[:, 256:512]
            # Y_re = sum Fr.T Gr + Fi.T Gi ; Y_im = sum Fr.T Gi - Fi.T Gr
            for k1c in range(2):
                frh = C[f"fr{k1c}"][:, 0:128]
                fih = C[f"fi{k1c}"][:, 0:128]
                nc.tensor.matmul(yre, frh, cre(g, k1c),
                                 start=(k1c == 0), stop=False)
                nc.tensor.matmul(yre, fih, cim(g, k1c),
                                 start=False, stop=(k1c == 1))
            for k1c in range(2):
                frh = C[f"fr{k1c}"][:, 0:128]
                nfih = C[f"nfi{k1c}"][:, 0:128]
                nc.tensor.matmul(yim, frh, cim(g, k1c),
                                 start=(k1c == 0), stop=False)
                nc.tensor.matmul(yim, nfih, cre(g, k1c),
                                 start=False, stop=(k1c == 1))

            ysb = ypool.tile([128, 512], F32, tag="ysb", name="ysb")
            nc.scalar.copy(ysb[:], yps[:])
            nc.sync.dma_start(yg[2 * p], ysb[:, 0:256])
            nc.sync.dma_start(yg[2 * p + 1], ysb[:, 256:512])

        # 4-deep software pipeline: every stage consumes data produced a
        # full step earlier, so no engine ever waits on an in-step evac.
        for p in range(NPAIR + 3):
            if p < NPAIR:
                emit_s1_tw1(p)
            if 1 <= p < NPAIR + 1:
                emit_s2_sp(p - 1)
            if 2 <= p < NPAIR + 2:
                emit_s3_tw2(p - 2)
            if p >= 3:
                emit_s4_out(p - 3)

    nc.compile()
    return nc


def _get_program(mm_dtype_name):
    if mm_dtype_name not in _PROGRAM_CACHE:
        _PROGRAM_CACHE[mm_dtype_name] = _build_program(mm_dtype_name)
    return _PROGRAM_CACHE[mm_dtype_name]


def kernel(x, filt):
    global LAST_RESULT
    assert x.shape == (B, TLEN) and x.dtype == np.float32
    nc = _get_program(MM_DTYPE)

    consts = dict(_static_consts())
    consts["spa"], consts["spb"] = _filter_spectrum(filt)

    in_maps = []
    for c in range(NCORES):
        m = dict(consts)
        m["x"] = np.ascontiguousarray(x[c * RPC:(c + 1) * RPC])
        in_maps.append(m)

    res = run_bass_kernel_spmd(
        nc, in_maps, core_ids=list(range(NCORES)),
        trace=bool(int(os.environ.get("EFF_TRACE", "0"))),
    )
    LAST_RESULT = res
    return np.concatenate([r["y"] for r in res.results], axis=0)
